# revision 1
# baseline (speedup 1.0000x reference)
# GAT 2-layer kernel for trn2 x8 — host prep + bass program + runner.
# Sharding: dst-node partition (graph parallel). Each core owns a contiguous
# 6272-node block and processes all edges into it; layer-1 node features are
# computed replicated; layer-2 node tables are exchanged via AllGather.
from contextlib import ExitStack

import numpy as np

import concourse.bass as bass
import concourse.bacc as bacc
import concourse.tile as tile
from concourse import mybir, library_config
from concourse.bass_utils import run_bass_kernel_spmd
from concourse.masks import make_identity

# ---- problem constants ----
N = 50000
DIN = 128
H1, C1 = 4, 32
C2 = 32
NCORE = 8

NODE_PAD = 50048          # 391 * 128
NT = NODE_PAD // 128      # 391 node tiles
CS = 6272                 # slots per core (49*128); node n -> core n//CS, rank n%CS
NMACRO = CS // 128        # 49
WIN = 32                  # dst-window width (psum base must be 32-aligned)
NW = 128 // WIN
EPT = 128                 # max edges per tile
GM = 3                    # macros per gather group

F1 = 132                  # layer-1 msg width: 4 head blocks of [32 feats | 1.0]
ROW1_H = 148              # f16 cols per table1 row (296 B)
ROW1_W = 74               # f32 cols
F2 = 33                   # layer-2 msg width: 32 feats + 1.0
ROW2_H = 40               # f16 cols per table2 row (80 B)
ROW2_W = 20
SHIFT1 = 6.0
SHIFT2 = 2.0

FP16 = mybir.dt.float16
FP32 = mybir.dt.float32
I32 = mybir.dt.int32
I16 = mybir.dt.int16

HA = 25088                # xT gather first-half rows
HB = NODE_PAD - HA        # 24960


def _phys1(n):
    return (n % 128) * NT + (n // 128)


def _phys2(core, slot):
    return core * CS + (slot % 128) * NMACRO + (slot // 128)


def _wrap16(a):
    n = len(a)
    assert n % 16 == 0
    out = np.empty((16, n // 16), np.int16)
    for k in range(n):
        out[k % 16, k // 16] = a[k]
    return out


def host_prep(inputs):
    """Canonical tile schedule shared by all cores + per-core blobs."""
    ei = np.asarray(inputs["edge_index"])
    src = np.concatenate([ei[0], np.arange(N, dtype=np.int64)]).astype(np.int64)
    dst = np.concatenate([ei[1], np.arange(N, dtype=np.int64)]).astype(np.int64)

    cores = []
    for c in range(NCORE):
        base = CS * c
        hi = min(base + CS, N)
        m = (dst >= base) & (dst < hi)
        s_c = src[m]
        r_c = (dst[m] - base).astype(np.int64)
        order = np.argsort(r_c, kind="stable")
        s_c = s_c[order]
        r_c = r_c[order]
        deg = np.bincount(r_c, minlength=CS)
        assert deg.max() <= EPT
        starts = np.zeros(CS + 1, np.int64)
        np.cumsum(deg, out=starts[1:])
        wt = {}
        for mac in range(NMACRO):
            for w in range(NW):
                lo = mac * 128 + w * WIN
                tl = []
                cur, cur_e = [], 0
                for r in range(lo, lo + WIN):
                    dd = int(deg[r])
                    if dd == 0:
                        continue
                    if cur_e + dd > EPT:
                        tl.append(cur)
                        cur, cur_e = [], 0
                    cur.append(r)
                    cur_e += dd
                if cur:
                    tl.append(cur)
                wt[(mac, w)] = tl
        cores.append(dict(s=s_c, starts=starts, wt=wt))

    tpw = np.zeros((NMACRO, NW), np.int32)
    for cc in cores:
        for (mac, w), tl in cc["wt"].items():
            tpw[mac, w] = max(tpw[mac, w], len(tl))
    sched = []  # (macro, window, first_of_window, last_of_window)
    for mac in range(NMACRO):
        for w in range(NW):
            nt = int(tpw[mac, w])
            assert nt >= 1
            for j in range(nt):
                sched.append((mac, w, j == 0, j == nt - 1))
    T = len(sched)

    per_core = []
    for c, cc in enumerate(cores):
        pat = np.zeros((128, T * WIN), np.float16)
        idx_s1 = np.zeros((128, T), np.int32)
        idx_d1 = np.zeros((128, T), np.int32)
        idx_s2 = np.zeros((128, T), np.int32)
        idx_d2 = np.zeros((128, T), np.int32)
        starts, s_c = cc["starts"], cc["s"]
        t = 0
        for mac in range(NMACRO):
            for w in range(NW):
                tl = cc["wt"][(mac, w)]
                for j in range(int(tpw[mac, w])):
                    if j < len(tl):
                        e = 0
                        for r in tl[j]:
                            for k in range(starts[r], starts[r + 1]):
                                sn = int(s_c[k])
                                pat[e, t * WIN + (r - mac * 128 - w * WIN)] = 1.0
                                idx_s1[e, t] = _phys1(sn)
                                idx_d1[e, t] = _phys1(CS * c + r)
                                idx_s2[e, t] = _phys2(sn // CS, sn % CS)
                                idx_d2[e, t] = _phys2(c, r)
                                e += 1
                    t += 1
        per_core.append(dict(pat=pat, idx_s1=idx_s1, idx_d1=idx_d1,
                             idx_s2=idx_s2, idx_d2=idx_d2))
    return sched, per_core


def make_in_maps(inputs, sched, per_core):
    x = np.asarray(inputs["x"], np.float32)
    xpad = np.zeros((NODE_PAD, DIN), np.float32)
    xpad[:N] = x
    W1 = np.asarray(inputs["W1"], np.float32)
    perm1 = np.zeros(128, np.int64)
    for cc_ in range(C1):
        for h in range(H1):
            perm1[cc_ * H1 + h] = h * C1 + cc_
    W1p = np.ascontiguousarray(W1[:, perm1])
    a_s1 = np.asarray(inputs["att_src1"], np.float32)
    a_d1 = np.asarray(inputs["att_dst1"], np.float32)
    attv1 = np.zeros((128, 8), np.float32)
    for h in range(H1):
        for cc_ in range(C1):
            attv1[cc_ * H1 + h, h] = a_s1[h, cc_]
            attv1[cc_ * H1 + h, 4 + h] = a_d1[h, cc_]
    W2 = np.asarray(inputs["W2"], np.float32)
    W2p = np.ascontiguousarray(W2[perm1, :])
    a_s2 = np.asarray(inputs["att_src2"], np.float32)[0]
    a_d2 = np.asarray(inputs["att_dst2"], np.float32)[0]
    attv2 = np.zeros((C2, 2), np.float32)
    attv2[:, 0] = a_s2
    attv2[:, 1] = a_d2
    assert not np.any(np.asarray(inputs["bias1"])) and \
        not np.any(np.asarray(inputs["bias2"])), "nonzero bias unsupported"

    in_maps = []
    for c in range(NCORE):
        pc = per_core[c]
        in_maps.append(dict(
            xpad=xpad, W1p=W1p, attv1=attv1, W2p=W2p, attv2=attv2,
            pat=pc["pat"], idx_s1=pc["idx_s1"], idx_d1=pc["idx_d1"],
            idx_s2=pc["idx_s2"], idx_d2=pc["idx_d2"],
        ))
    return in_maps


def build_program(sched, debug=False):
    T = len(sched)
    nc = bacc.Bacc("TRN2", target_bir_lowering=False, debug=False, num_devices=NCORE)
    tc = tile.TileContext(nc)

    xpad_d = nc.dram_tensor("xpad", [NODE_PAD, DIN], FP32, kind="ExternalInput")
    W1p_d = nc.dram_tensor("W1p", [128, 128], FP32, kind="ExternalInput")
    attv1_d = nc.dram_tensor("attv1", [128, 8], FP32, kind="ExternalInput")
    W2p_d = nc.dram_tensor("W2p", [128, C2], FP32, kind="ExternalInput")
    attv2_d = nc.dram_tensor("attv2", [C2, 2], FP32, kind="ExternalInput")
    pat_d = nc.dram_tensor("pat", [128, T * WIN], FP16, kind="ExternalInput")
    idx_s1_d = nc.dram_tensor("idx_s1", [128, T], I32, kind="ExternalInput")
    idx_d1_d = nc.dram_tensor("idx_d1", [128, T], I32, kind="ExternalInput")
    idx_s2_d = nc.dram_tensor("idx_s2", [128, T], I32, kind="ExternalInput")
    idx_d2_d = nc.dram_tensor("idx_d2", [128, T], I32, kind="ExternalInput")
    out2_d = nc.dram_tensor("out2", [CS, C2], FP32, kind="ExternalOutput")

    tab1_d = nc.dram_tensor("tab1i", [NODE_PAD, ROW1_H], FP16)
    if debug:
        dbg_tab1 = nc.dram_tensor("dbg_tab1", [256, ROW1_H], FP16, kind="ExternalOutput")
        dbg_g1 = nc.dram_tensor("dbg_g1", [128, 280], FP16, kind="ExternalOutput")
        dbg_u = nc.dram_tensor("dbg_u", [128, 8], FP32, kind="ExternalOutput")
        dbg_msg = nc.dram_tensor("dbg_msg", [128, 2 * F1], FP16, kind="ExternalOutput")
        dbg_helu = nc.dram_tensor("dbg_helu", [128, 128], FP16, kind="ExternalOutput")
        dbg_ht = nc.dram_tensor("dbg_ht", [128, 128], FP16, kind="ExternalOutput")
        dbg_tb2g = nc.dram_tensor("dbg_tb2g", [256, ROW2_H], FP16, kind="ExternalOutput")
        dbg_agg = nc.dram_tensor("dbg_agg", [128, F1], FP32, kind="ExternalOutput")
        dbg_g2 = nc.dram_tensor("dbg_g2", [128, 2 * ROW2_H], FP16, kind="ExternalOutput")
        dbg_u2 = nc.dram_tensor("dbg_u2", [128, 8], FP32, kind="ExternalOutput")
        dbg_m2 = nc.dram_tensor("dbg_m2", [128, 2 * 34], FP16, kind="ExternalOutput")
        dbg_ag2 = nc.dram_tensor("dbg_ag2", [128, F2], FP32, kind="ExternalOutput")
        dbg_ad2 = nc.dram_tensor("dbg_ad2", [128, 4], FP32, kind="ExternalOutput")
    tb2l_d = nc.dram_tensor("tb2li", [CS, ROW2_H], FP16)
    tb2g_d = nc.dram_tensor("tb2gi", [NCORE * CS, ROW2_H], FP16, addr_space="Shared")

    # group boundaries (GM macros per gather group)
    mstart = [0] * (NMACRO + 1)
    for i, (mac, w, fw, lw) in enumerate(sched):
        mstart[mac + 1] = i + 1
    groups = []
    for g0 in range(0, NMACRO, GM):
        g1_ = min(g0 + GM, NMACRO)
        groups.append((mstart[g0], mstart[g1_], g0, g1_))
    GT_MAX = max(t1 - t0 for t0, t1, _, _ in groups)

    with tc, ExitStack() as ctx:
        cc_sem = ctx.enter_context(nc.semaphore("ccsem"))
        const = ctx.enter_context(tc.tile_pool(name="const", bufs=1))

        w1_sb = const.tile([128, 128], FP32)
        nc.sync.dma_start(w1_sb[:], W1p_d.ap())
        attv1_sb = const.tile([128, 8], FP32)
        nc.sync.dma_start(attv1_sb[:], attv1_d.ap())
        w2_sb = const.tile([128, C2], FP32)
        nc.sync.dma_start(w2_sb[:], W2p_d.ap())
        attv2_sb = const.tile([C2, 2], FP32)
        nc.sync.dma_start(attv2_sb[:], attv2_d.ap())

        ident = const.tile([128, 128], FP32)
        make_identity(nc, ident[:])
        ident16 = const.tile([128, 128], FP16)
        make_identity(nc, ident16[:])
        nsh1 = const.tile([128, 1], FP32)
        nc.gpsimd.memset(nsh1[:], -SHIFT1)
        nsh2 = const.tile([128, 1], FP32)
        nc.gpsimd.memset(nsh2[:], -SHIFT2)

        with tc.tile_pool(name="ps0", bufs=1, space="PSUM") as ps0:
            w1t_ps = ps0.tile([128, 128], FP32, space="PSUM")
            nc.tensor.transpose(w1t_ps[:], w1_sb[:], ident[:])
            w1t_sb = const.tile([128, 128], FP32)
            nc.vector.tensor_copy(w1t_sb[:], w1t_ps[:])
            wat1_ps = ps0.tile([128, 8], FP32, space="PSUM")
            nc.tensor.matmul(wat1_ps[:], w1t_sb[:], attv1_sb[:], start=True, stop=True)
            rhs1 = const.tile([128, 136], FP16)
            nc.vector.tensor_copy(rhs1[:, 0:128], w1_sb[:])
            nc.vector.tensor_copy(rhs1[:, 128:136], wat1_ps[:])
            w2t_ps = ps0.tile([C2, 128], FP32, space="PSUM")
            nc.tensor.transpose(w2t_ps[:], w2_sb[:], ident[:])
            w2t_sb = const.tile([C2, 128], FP32)
            nc.vector.tensor_copy(w2t_sb[:], w2t_ps[:])
            wat2_ps = ps0.tile([128, 2], FP32, space="PSUM")
            nc.tensor.matmul(wat2_ps[:], w2t_sb[:], attv2_sb[:], start=True, stop=True)
            rhs2 = const.tile([128, C2 + 2], FP16)
            nc.vector.tensor_copy(rhs2[:, 0:C2], w2_sb[:])
            nc.vector.tensor_copy(rhs2[:, C2:C2 + 2], wat2_ps[:])

        # ---------------- phase A: xT (PE transpose) + table1 ----------------
        xpad_v = xpad_d.ap().rearrange("(t p) c -> p t c", p=128)  # [128, NT, 128]
        with tc.tile_pool(name="xc", bufs=2) as xcp, \
             tc.tile_pool(name="xt", bufs=4) as xtp, \
             tc.tile_pool(name="pa_ps", bufs=3, space="PSUM") as paps, \
             tc.tile_pool(name="pa_st", bufs=3) as past:
            CH = 4
            for nt0 in range(0, NT, CH):
                nch = min(CH, NT - nt0)
                xc = xcp.tile([128, CH * 128], FP32, tag="xc")
                nc.sync.dma_start(
                    xc[:, 0:nch * 128].rearrange("p (t c) -> p t c", c=128),
                    xpad_v[:, nt0:nt0 + nch, :])
                st = past.tile([128, CH * ROW1_H], FP16, tag="stage")
                stf = st[:].bitcast(FP32)
                nc.vector.memset(
                    st[:].rearrange("p (t c) -> p t c", c=ROW1_H)[:, 0:nch, 128:132],
                    1.0)
                for k in range(nch):
                    xtps = paps.tile([128, 128], FP32, space="PSUM", tag="xtp")
                    nc.tensor.transpose(xtps[:], xc[:, k * 128:(k + 1) * 128],
                                        ident[:])
                    xts = xtp.tile([128, 128], FP16, tag="xts")
                    nc.scalar.activation(xts[:], xtps[:],
                                         mybir.ActivationFunctionType.Copy)
                    h1ps = paps.tile([128, 136], FP32, space="PSUM", tag="h1")
                    nc.tensor.matmul(h1ps[:], xts[:], rhs1[:], start=True, stop=True)
                    nc.scalar.activation(
                        st[:, k * ROW1_H: k * ROW1_H + 128], h1ps[:, 0:128],
                        mybir.ActivationFunctionType.Copy)
                    nc.vector.tensor_copy(
                        stf[:, k * ROW1_W + 66: k * ROW1_W + 74], h1ps[:, 128:136])
                dst_ap = tab1_d.ap().rearrange("(p t) c -> p t c", p=128)
                nc.sync.dma_start(
                    dst_ap[:, nt0:nt0 + nch, :],
                    st[:].rearrange("p (t c) -> p t c", c=ROW1_H)[:, 0:nch, :])

        tc.strict_bb_all_engine_barrier()

        # ---------------- resident edge data ----------------
        epool = ctx.enter_context(tc.tile_pool(name="edata", bufs=1))
        pat_sb = epool.tile([128, T * WIN], FP16)
        nc.sync.dma_start(pat_sb[:], pat_d.ap())
        ixs1 = epool.tile([128, T], I32)
        nc.sync.dma_start(ixs1[:], idx_s1_d.ap())
        ixd1 = epool.tile([128, T], I32)
        nc.sync.dma_start(ixd1[:], idx_d1_d.ap())
        ixs2 = epool.tile([128, T], I32)
        nc.sync.dma_start(ixs2[:], idx_s2_d.ap())
        ixd2 = epool.tile([128, T], I32)
        nc.sync.dma_start(ixd2[:], idx_d2_d.ap())
        htpool = ctx.enter_context(tc.tile_pool(name="ht", bufs=1))
        ht = htpool.tile([128, CS], FP16)

        # ---------------- phase B: layer-1 edges ----------------
        tab1_h = tab1_d.ap()
        tab1_f = tab1_d.ap().bitcast(FP32)
        with tc.tile_pool(name="g1", bufs=2) as g1p, \
             tc.tile_pool(name="zu", bufs=2) as zup, \
             tc.tile_pool(name="msgp", bufs=2) as msgp, \
             tc.tile_pool(name="aggp", bufs=4, space="PSUM") as aggp, \
             tc.tile_pool(name="htps", bufs=2, space="PSUM") as htpsp, \
             tc.tile_pool(name="hn", bufs=2) as hnp:
            for (t0, t1, m0, m1) in groups:
                gt = t1 - t0
                g1 = g1p.tile([128, GT_MAX * 140], FP16, tag="g1")
                adg = zup.tile([128, GT_MAX * 4], FP32, tag="ad")
                for t in range(t0, t1):
                    j = t - t0
                    nc.gpsimd.indirect_dma_start(
                        out=g1[:, j * 140:(j + 1) * 140], out_offset=None, in_=tab1_h,
                        in_offset=bass.IndirectOffsetOnAxis(ap=ixs1[:, t:t + 1], axis=0))
                    nc.gpsimd.indirect_dma_start(
                        out=adg[:, j * 4:(j + 1) * 4], out_offset=None, in_=tab1_f,
                        in_offset=bass.IndirectOffsetOnAxis(ap=ixd1[:, t:t + 1], axis=0),
                        element_offset=70)
                g1f = g1[:].bitcast(FP32)
                z = zup.tile([128, GT_MAX * 4], FP32, tag="z")
                nc.vector.tensor_tensor(
                    out=z[:, 0:gt * 4].rearrange("p (t h) -> p t h", h=4),
                    in0=g1f[:, 0:gt * 70].rearrange("p (t c) -> p t c", c=70)[:, :, 66:70],
                    in1=adg[:, 0:gt * 4].rearrange("p (t h) -> p t h", h=4),
                    op=mybir.AluOpType.add)
                zs = zup.tile([128, GT_MAX * 4], FP32, tag="zs")
                nc.vector.tensor_scalar_mul(zs[:, 0:gt * 4], z[:, 0:gt * 4], 0.2)
                nc.vector.tensor_tensor(out=z[:, 0:gt * 4], in0=z[:, 0:gt * 4],
                                        in1=zs[:, 0:gt * 4], op=mybir.AluOpType.max)
                u = zup.tile([128, GT_MAX * 4], FP16, tag="u")
                nc.scalar.activation(u[:, 0:gt * 4], z[:, 0:gt * 4],
                                     mybir.ActivationFunctionType.Exp, bias=nsh1[:])
                msg = msgp.tile([128, GT_MAX * F1], FP16, tag="msg")
                nc.vector.tensor_tensor(
                    out=msg[:, 0:gt * F1].rearrange("p (t c h) -> p t c h", c=33, h=4),
                    in0=g1[:, 0:gt * 140].rearrange("p (t c) -> p t c", c=140)[:, :, 0:132]
                        .rearrange("p t (c h) -> p t c h", h=4),
                    in1=u[:, 0:gt * 4].rearrange("p (t h) -> p t h", h=4)
                        .unsqueeze(2).broadcast_to([128, gt, 33, 4]),
                    op=mybir.AluOpType.mult)
                if debug and t0 == 0:
                    nc.sync.dma_start(dbg_g1.ap(), g1[:, 0:280])
                    udbg = zup.tile([128, 8], FP32, tag="udbg")
                    nc.vector.tensor_copy(udbg[:], u[:, 0:8])
                    nc.sync.dma_start(dbg_u.ap(), udbg[:])
                    nc.sync.dma_start(dbg_msg.ap(), msg[:, 0:2 * F1])
                for mac in range(m0, m1):
                    agg = aggp.tile([128, F1], FP32, space="PSUM", tag="agg")
                    for t in range(mstart[mac], mstart[mac + 1]):
                        _, w, fw, lw = sched[t]
                        nc.tensor.matmul(
                            agg[w * WIN:(w + 1) * WIN, :],
                            pat_sb[:, t * WIN:(t + 1) * WIN],
                            msg[:, (t - t0) * F1:(t - t0 + 1) * F1],
                            start=fw, stop=lw, tile_position=(0, w * WIN))
                    if debug and mac == 0:
                        aggd = hnp.tile([128, F1], FP32, tag="aggd")
                        nc.vector.tensor_copy(aggd[:], agg[:])
                        nc.sync.dma_start(dbg_agg.ap(), aggd[:])
                    r = hnp.tile([128, 4], FP32, tag="r")
                    nc.vector.reciprocal(r[:], agg[:, 128:132])
                    hn = hnp.tile([128, 128], FP16, tag="hn")
                    nc.vector.tensor_tensor(
                        out=hn[:].rearrange("p (c h) -> p c h", h=4),
                        in0=agg[:, 0:128].rearrange("p (c h) -> p c h", h=4),
                        in1=r[:].unsqueeze(1).broadcast_to([128, 32, 4]),
                        op=mybir.AluOpType.mult)
                    a = hnp.tile([128, 128], FP16, tag="elua")
                    nc.vector.tensor_scalar_min(a[:], hn[:], 0.0)
                    e = hnp.tile([128, 128], FP16, tag="elue")
                    nc.scalar.activation(e[:], a[:], mybir.ActivationFunctionType.Exp)
                    em1 = hnp.tile([128, 128], FP16, tag="eluem")
                    nc.vector.tensor_scalar_add(em1[:], e[:], -1.0)
                    helu = hnp.tile([128, 128], FP16, tag="helu")
                    nc.vector.tensor_tensor(out=helu[:], in0=hn[:], in1=em1[:],
                                            op=mybir.AluOpType.max)
                    htps = htpsp.tile([128, 128], FP16, space="PSUM", tag="htp")
                    nc.tensor.transpose(htps[:], helu[:], ident16[:])
                    nc.scalar.activation(
                        ht[:, mac * 128:(mac + 1) * 128], htps[:],
                        mybir.ActivationFunctionType.Copy)
                    if debug and mac == 0:
                        nc.sync.dma_start(dbg_helu.ap(), helu[:])

        tc.strict_bb_all_engine_barrier()

        # ---------------- phase C: q / table2 + AllGather ----------------
        with tc.tile_pool(name="pc_ps", bufs=6, space="PSUM") as pcps, \
             tc.tile_pool(name="pc_st", bufs=3) as pcst:
            CH2 = 8
            for sc0 in range(0, NMACRO, CH2):
                nch = min(CH2, NMACRO - sc0)
                st2 = pcst.tile([128, CH2 * ROW2_H], FP16, tag="st2")
                st2f = st2[:].bitcast(FP32)
                nc.vector.memset(
                    st2[:].rearrange("p (t c) -> p t c", c=ROW2_H)[:, 0:nch, C2:C2 + 1],
                    1.0)
                for k in range(nch):
                    scc = sc0 + k
                    qps = pcps.tile([128, C2 + 2], FP32, space="PSUM", tag="q")
                    nc.tensor.matmul(qps[:], ht[:, scc * 128:(scc + 1) * 128],
                                     rhs2[:], start=True, stop=True)
                    nc.scalar.activation(
                        st2[:, k * ROW2_H: k * ROW2_H + C2], qps[:, 0:C2],
                        mybir.ActivationFunctionType.Copy)
                    nc.vector.tensor_copy(
                        st2f[:, k * ROW2_W + 17: k * ROW2_W + 19], qps[:, C2:C2 + 2])
                dst2 = tb2l_d.ap().rearrange("(p t) c -> p t c", p=128)
                nc.sync.dma_start(
                    dst2[:, sc0:sc0 + nch, :],
                    st2[:].rearrange("p (t c) -> p t c", c=ROW2_H)[:, 0:nch, :])

        if debug:
            nc.sync.dma_start(dbg_tab1.ap(), tab1_d.ap()[0:256, :])
            nc.sync.dma_start(dbg_ht.ap(), ht[:, 0:128])
        tc.strict_bb_all_engine_barrier()
        with tc.tile_critical():
            nc.gpsimd.collective_compute(
                "AllGather", mybir.AluOpType.bypass,
                replica_groups=[list(range(NCORE))],
                ins=[tb2l_d.ap().opt()],
                outs=[tb2g_d.ap().opt()],
            ).then_inc(cc_sem)
            nc.gpsimd.wait_ge(cc_sem, 1)
        tc.strict_bb_all_engine_barrier()

        if debug:
            nc.sync.dma_start(dbg_tb2g.ap(), tb2g_d.ap()[CS:CS + 256, :])
        # ---------------- phase D: layer-2 edges ----------------
        tab2_h = tb2g_d.ap()
        tab2_f = tb2g_d.ap().bitcast(FP32)
        with tc.tile_pool(name="g2", bufs=2) as g2p, \
             tc.tile_pool(name="zu2", bufs=2) as zup2, \
             tc.tile_pool(name="msg2", bufs=2) as msgp2, \
             tc.tile_pool(name="agg2", bufs=6, space="PSUM") as aggp2, \
             tc.tile_pool(name="o2", bufs=2) as o2p:
            for (t0, t1, m0, m1) in groups:
                gt = t1 - t0
                g2 = g2p.tile([128, GT_MAX * ROW2_H], FP16, tag="g2")
                ad2 = zup2.tile([128, GT_MAX], FP32, tag="ad2")
                for t in range(t0, t1):
                    j = t - t0
                    nc.gpsimd.indirect_dma_start(
                        out=g2[:, j * ROW2_H:(j + 1) * ROW2_H], out_offset=None,
                        in_=tab2_h,
                        in_offset=bass.IndirectOffsetOnAxis(ap=ixs2[:, t:t + 1], axis=0))
                    nc.gpsimd.indirect_dma_start(
                        out=ad2[:, j:j + 1], out_offset=None, in_=tab2_f,
                        in_offset=bass.IndirectOffsetOnAxis(ap=ixd2[:, t:t + 1], axis=0),
                        element_offset=18)
                g2f = g2[:].bitcast(FP32)
                z2 = zup2.tile([128, GT_MAX], FP32, tag="z2")
                nc.vector.tensor_tensor(
                    out=z2[:, 0:gt].unsqueeze(2),
                    in0=g2f[:, 0:gt * 20].rearrange("p (t c) -> p t c", c=20)[:, :, 17:18],
                    in1=ad2[:, 0:gt].unsqueeze(2),
                    op=mybir.AluOpType.add)
                zs2 = zup2.tile([128, GT_MAX], FP32, tag="zs2")
                nc.vector.tensor_scalar_mul(zs2[:, 0:gt], z2[:, 0:gt], 0.2)
                nc.vector.tensor_tensor(out=z2[:, 0:gt], in0=z2[:, 0:gt],
                                        in1=zs2[:, 0:gt], op=mybir.AluOpType.max)
                u2 = zup2.tile([128, GT_MAX * 2], FP16, tag="u2")
                u2v = u2[:].rearrange("p (t j) -> p t j", j=2)
                nc.scalar.activation(u2v[:, 0:gt, 0:1], z2[:, 0:gt].unsqueeze(2),
                                     mybir.ActivationFunctionType.Exp, bias=nsh2[:])
                nc.vector.tensor_copy(u2v[:, 0:gt, 1:2], u2v[:, 0:gt, 0:1])
                msg2 = msgp2.tile([128, GT_MAX * 34], FP16, tag="m2")
                nc.vector.tensor_tensor(
                    out=msg2[:, 0:gt * 34].rearrange("p (t c j) -> p t c j", c=17, j=2),
                    in0=g2[:, 0:gt * ROW2_H].rearrange("p (t c) -> p t c", c=ROW2_H)
                        [:, :, 0:34].rearrange("p t (c j) -> p t c j", j=2),
                    in1=u2v[:, 0:gt, :].unsqueeze(2).broadcast_to([128, gt, 17, 2]),
                    op=mybir.AluOpType.mult)
                if debug and t0 == 0:
                    nc.sync.dma_start(dbg_g2.ap(), g2[:, 0:2 * ROW2_H])
                    u2d = zup2.tile([128, 8], FP32, tag="u2d")
                    nc.vector.tensor_copy(u2d[:], u2[:, 0:8])
                    nc.sync.dma_start(dbg_u2.ap(), u2d[:])
                    nc.sync.dma_start(dbg_m2.ap(), msg2[:, 0:2 * 34])
                    ad2d = zup2.tile([128, 4], FP32, tag="ad2d")
                    nc.vector.tensor_copy(ad2d[:], ad2[:, 0:4])
                    nc.sync.dma_start(dbg_ad2.ap(), ad2d[:])
                for mac in range(m0, m1):
                    agg2 = aggp2.tile([128, F2], FP32, space="PSUM", tag="ag2")
                    for t in range(mstart[mac], mstart[mac + 1]):
                        _, w, fw, lw = sched[t]
                        nc.tensor.matmul(
                            agg2[w * WIN:(w + 1) * WIN, :],
                            pat_sb[:, t * WIN:(t + 1) * WIN],
                            msg2[:, (t - t0) * 34:(t - t0) * 34 + F2],
                            start=fw, stop=lw, tile_position=(0, w * WIN))
                    if debug and mac == 0:
                        ag2d = o2p.tile([128, F2], FP32, tag="ag2d")
                        nc.vector.tensor_copy(ag2d[:], agg2[:])
                        nc.sync.dma_start(dbg_ag2.ap(), ag2d[:])
                    r2 = o2p.tile([128, 1], FP32, tag="r2")
                    nc.vector.reciprocal(r2[:], agg2[:, C2:C2 + 1])
                    o2 = o2p.tile([128, C2], FP32, tag="o2")
                    nc.vector.tensor_tensor(
                        out=o2[:], in0=agg2[:, 0:C2],
                        in1=r2[:].broadcast_to([128, C2]),
                        op=mybir.AluOpType.mult)
                    nc.sync.dma_start(out2_d.ap()[mac * 128:(mac + 1) * 128, :], o2[:])

    nc.compile()
    return nc


_CACHE = {}


def run(inputs, trace=False, debug=False):
    sched, per_core = host_prep(inputs)
    in_maps = make_in_maps(inputs, sched, per_core)
    key = (len(sched), debug)
    if key not in _CACHE:
        _CACHE[key] = build_program(sched, debug=debug)
    nc = _CACHE[key]
    res = run_bass_kernel_spmd(nc, in_maps, core_ids=list(range(NCORE)), trace=trace)
    outs = [r["out2"] for r in res.results]
    out = np.zeros((N, C2), np.float32)
    for c in range(NCORE):
        lo = c * CS
        hi = min(lo + CS, N)
        out[lo:hi] = outs[c][: hi - lo]
    return out, res


def kernel(**inputs):
    """Full-input GAT kernel: shards across 8 NeuronCores internally."""
    import numpy as _np
    out, _res = run(inputs)
    return out.astype(_np.float32)



# revision 11
# speedup vs baseline: 2.6755x; 2.6755x over previous
# 2-layer GAT on trn2 x8 — dst-partitioned (graph parallel).
#
# Layer 1 avoids all device-side gathers: the host pre-gathers x rows into
# dst-sorted edge-slot order (pure layout prep), and the device projects
# per-edge features with PE matmuls (h | att-src dots in one pass). Per-edge
# dst-attention dots come from stacked one-hot transpose matmuls against the
# per-node dot table. Aggregation (numerator + softmax denominator) is the
# baseline's one-hot window matmul.
#
# Layer 2 gathers the AllGather'd per-node table (256B rows: h2 | 1 | u2)
# with batched gpsimd dma_gather (2 calls per 3-macro group, lo/hi table
# halves since gather indices are int16), replacing the baseline's ~4000
# per-tile INDIRECT1D descents (~1.1us fixed cost each).
#
# This file is self-contained: it inlines host schedule construction
# (host_prep/host_blobs equivalents) and hardcodes all shapes.
from contextlib import ExitStack

import numpy as np

import concourse.bass as bass
import concourse.bacc as bacc
import concourse.tile as tile
from concourse import mybir
from concourse.bass_utils import run_bass_kernel_spmd
from concourse.masks import make_identity

# ---- problem constants ----
N = 50000
DIN = 128
H1, C1 = 4, 32
C2 = 32
NCORE = 8
CS = 6272
NMACRO = CS // 128        # 49
WIN = 32
NW = 4
EPT = 128
GM_B = 2                  # phase-B group: macros
GM_D = 3                  # phase-D group: macros
HA2 = 25088               # tab2 lo half (cores 0-3)
ROW1 = 136                # proj psum/staging row: h(128) | ones(4) | u_src(4)
F1 = 132
F2 = 33
ROW2 = 128                # tab2 row: h2(32) | 1.0 | u2 | pad
SHIFT1, SHIFT2 = 6.0, 2.0

FP16 = mybir.dt.float16
FP32 = mybir.dt.float32
I16 = mybir.dt.int16

NPAD = NCORE * CS         # 50176


def _phys2(n):
    core, slot = divmod(int(n), CS)
    return core * CS + (slot % 128) * NMACRO + slot // 128


def _wrap16(a):
    a = np.asarray(a, np.int64)
    n = len(a)
    cols = max(1, (n + 15) // 16)
    out = np.full((16, cols), -1, np.int16)
    out[np.arange(n) % 16, np.arange(n) // 16] = a.astype(np.int16)
    return np.tile(out, (8, 1))


# ======================== host schedule + blobs =========================
def host_prep(inputs):
    ei = np.asarray(inputs["edge_index"])
    # self-loops for all nodes incl. pads (keeps every dst row non-empty so
    # no inf/nan reciprocals leak through the dst-dot matmuls)
    src_all = np.concatenate([ei[0].astype(np.int64),
                              np.arange(NPAD, dtype=np.int64)])
    dst_all = np.concatenate([ei[1].astype(np.int64),
                              np.arange(NPAD, dtype=np.int64)])

    cores = []
    for c in range(NCORE):
        base = c * CS
        m = (dst_all >= base) & (dst_all < base + CS)
        s_c = src_all[m]
        r_c = dst_all[m] - base
        order = np.argsort(r_c, kind="stable")
        s_c, r_c = s_c[order], r_c[order]
        deg = np.bincount(r_c, minlength=CS)
        assert deg.max() <= EPT
        cores.append(dict(s=s_c, r=r_c, deg=deg))

    # ---- phase B schedule: window-pure tiles, padded to full stacks ----
    tpw = np.zeros((NMACRO, NW), np.int64)
    core_wt = []
    for c in range(NCORE):
        deg = cores[c]["deg"]
        wt = {}
        for mac in range(NMACRO):
            for w in range(NW):
                lo = mac * 128 + w * WIN
                tl, cur, cur_e = [], [], 0
                for r in range(lo, lo + WIN):
                    dd = int(deg[r])
                    if dd == 0:
                        continue
                    if cur_e + dd > EPT:
                        tl.append(cur)
                        cur, cur_e = [], 0
                    cur.append(r)
                    cur_e += dd
                if cur:
                    tl.append(cur)
                wt[(mac, w)] = tl
                tpw[mac, w] = max(tpw[mac, w], len(tl))
        core_wt.append(wt)

    schedB, nstack, stackB = [], [], []
    for mac in range(NMACRO):
        ns = int(tpw[mac].max())
        nstack.append(ns)
        for s in range(ns):
            stackB.append((mac, s, len(schedB)))
            for w in range(NW):
                schedB.append((mac, w, s))
    T1, NST = len(schedB), len(stackB)

    macB0 = np.zeros(NMACRO + 1, np.int64)
    for i, (mac, w, s) in enumerate(schedB):
        macB0[mac + 1] = i + 1
    st0 = np.zeros(NMACRO + 1, np.int64)
    for i, (mac, s, t0) in enumerate(stackB):
        st0[mac + 1] = i + 1
    groupsB = []
    for g0 in range(0, NMACRO, GM_B):
        g1 = min(g0 + GM_B, NMACRO)
        groupsB.append((int(macB0[g0]), int(macB0[g1]), g0, g1,
                       int(st0[g0]), int(st0[g1])))
    GTB = max(t1 - t0 for t0, t1, *_ in groupsB)
    GSB = max(s1 - s0 for *_, s0, s1 in groupsB)

    perB = []
    for c in range(NCORE):
        s_c, deg = cores[c]["s"], cores[c]["deg"]
        starts = np.zeros(CS + 1, np.int64)
        np.cumsum(deg, out=starts[1:])
        wt = core_wt[c]
        xet_src = np.full((128, T1), -1, np.int64)
        pat1 = np.zeros((128, T1 * WIN), np.float16)
        for t, (mac, w, s) in enumerate(schedB):
            tl = wt[(mac, w)]
            if s >= len(tl):
                continue
            e = 0
            for r in tl[s]:
                for k in range(starts[r], starts[r + 1]):
                    xet_src[e, t] = s_c[k]
                    pat1[e, t * WIN + (r - mac * 128 - w * WIN)] = 1.0
                    e += 1
        patT1 = np.zeros((128, NST * 128), np.float16)
        for i, (mac, s, t0) in enumerate(stackB):
            for w in range(NW):
                blk = pat1[:, (t0 + w) * WIN:(t0 + w + 1) * WIN]
                patT1[32 * w:32 * (w + 1), i * 128:(i + 1) * 128] = blk.T
        perB.append(dict(xet_src=xet_src, pat1=pat1, patT1=patT1))

    bdm = np.zeros((128, 16), np.float16)
    for w in range(NW):
        bdm[32 * w:32 * (w + 1), 4 * w:4 * w + 4] = 1.0

    # ---- phase D schedule: dense tiles, lo/hi per group ----
    groupsD = [(g0, min(g0 + GM_D, NMACRO)) for g0 in range(0, NMACRO, GM_D)]
    ND = len(groupsD)
    core_lohi = []
    for c in range(NCORE):
        s_c, r_c = cores[c]["s"], cores[c]["r"]
        ent = []
        for (g0, g1) in groupsD:
            m = (r_c >= g0 * 128) & (r_c < g1 * 128)
            sg, rg = s_c[m], r_c[m]
            is_lo = np.array([_phys2(s) < HA2 for s in sg], bool) \
                if len(sg) else np.zeros(0, bool)
            ent.append((sg[is_lo], rg[is_lo], sg[~is_lo], rg[~is_lo]))
        core_lohi.append(ent)

    nloM = [max(len(core_lohi[c][g][0]) for c in range(NCORE)) for g in range(ND)]
    nhiM = [max(len(core_lohi[c][g][2]) for c in range(NCORE)) for g in range(ND)]
    koff = [(n + 127) // 128 for n in nloM]
    ntileD = [koff[g] + (nhiM[g] + 127) // 128 for g in range(ND)]
    T2 = int(np.sum(ntileD))
    tbase = np.zeros(ND + 1, np.int64)
    for g in range(ND):
        tbase[g + 1] = tbase[g] + ntileD[g]

    slot_r = np.full((NCORE, 128, T2), -1, np.int64)
    slot_s = np.full((NCORE, 128, T2), -1, np.int64)
    for c in range(NCORE):
        for g in range(ND):
            slo, rlo, shi, rhi = core_lohi[c][g]
            t0 = int(tbase[g])
            jj = np.arange(len(slo))
            slot_r[c, jj % 128, t0 + jj // 128] = rlo
            slot_s[c, jj % 128, t0 + jj // 128] = slo
            hb_ = t0 + koff[g]
            jj = np.arange(len(shi))
            slot_r[c, jj % 128, hb_ + jj // 128] = rhi
            slot_s[c, jj % 128, hb_ + jj // 128] = shi

    segW, segM = [], []
    segW_g0, segM_g0 = [0], [0]
    for g in range(ND):
        t0, t1 = int(tbase[g]), int(tbase[g + 1])
        winset, macset = {}, {}
        for t in range(t0, t1):
            rr = slot_r[:, :, t]
            rr = rr[rr >= 0]
            if len(rr) == 0:
                continue
            for wv in sorted(set(int(r) // WIN for r in rr)):
                winset.setdefault(wv, []).append(t)
            for mv in sorted(set(int(r) // 128 for r in rr)):
                macset.setdefault(mv, []).append(t)
        for wv, ts in sorted(winset.items()):
            for i, t in enumerate(ts):
                segW.append((t, wv // NW, wv % NW, i == 0, i == len(ts) - 1))
        # ed2 accumulates per TILE across its macros: flags per tile.
        # Fully-pad tiles get a dummy seg (zero patT2) so their ed2 psum
        # column is written (avoids stale-psum inf/nan reaching exp()).
        for t in range(t0, t1):
            rr = slot_r[:, :, t]
            rr = rr[rr >= 0]
            ms = sorted(set(int(r) // 128 for r in rr)) \
                if len(rr) else [groupsD[g][0]]
            for k, mv in enumerate(ms):
                segM.append((t, mv, k == 0, k == len(ms) - 1))
        segW_g0.append(len(segW))
        segM_g0.append(len(segM))
    NSW, NSM = len(segW), len(segM)

    perD = []
    glo_off, ghi_off = [], []
    for c in range(NCORE):
        pat2 = np.zeros((128, NSW * WIN), np.float16)
        patT2 = np.zeros((128, NSM * 128), np.float16)
        for i, (t, mac, w, st, sp) in enumerate(segW):
            rr = slot_r[c, :, t]
            sel = (rr >= 0) & (rr // WIN == mac * NW + w)
            for p in np.nonzero(sel)[0]:
                pat2[p, i * WIN + int(rr[p]) - (mac * NW + w) * WIN] = 1.0
        for i, (t, mac, st, sp) in enumerate(segM):
            rr = slot_r[c, :, t]
            sel = (rr >= 0) & (rr // 128 == mac)
            for p in np.nonzero(sel)[0]:
                patT2[int(rr[p]) - mac * 128, i * 128 + p] = 1.0
        ilo_cols = sum(ntileD[g] * 8 for g in range(ND))
        ihi_cols = sum((ntileD[g] - koff[g]) * 8 for g in range(ND))
        ilo = np.full((128, ilo_cols), -1, np.int16)
        ihi = np.zeros((128, ihi_cols), np.int16)
        olo = ohi = 0
        glo_off, ghi_off = [], []
        for g in range(ND):
            slo = core_lohi[c][g][0]
            shi = core_lohi[c][g][2]
            nlo_call = ntileD[g] * 128
            nhi_call = (ntileD[g] - koff[g]) * 128
            a = np.full(nlo_call, -1, np.int64)
            a[: koff[g] * 128] = 0
            if len(slo):
                a[: len(slo)] = [_phys2(s) for s in slo]
            b = np.zeros(nhi_call, np.int64)
            if len(shi):
                b[: len(shi)] = [_phys2(s) - HA2 for s in shi]
            ilo[:, olo: olo + nlo_call // 16] = _wrap16(a)
            ihi[:, ohi: ohi + nhi_call // 16] = _wrap16(b)
            glo_off.append(olo)
            ghi_off.append(ohi)
            olo += nlo_call // 16
            ohi += nhi_call // 16
        perD.append(dict(pat2=pat2, patT2=patT2, ilo=ilo, ihi=ihi))

    return dict(
        schedB=schedB, stackB=stackB, groupsB=groupsB, T1=T1, NST=NST,
        GTB=GTB, GSB=GSB, nstack=nstack, bdm=bdm,
        groupsD=groupsD, ntileD=ntileD, koff=koff, tbase=tbase, T2=T2,
        segW=segW, segM=segM, NSW=NSW, NSM=NSM,
        segW_g0=segW_g0, segM_g0=segM_g0,
        glo_off=glo_off, ghi_off=ghi_off,
        ilo_cols=ilo_cols, ihi_cols=ihi_cols,
    ), perB, perD


def make_in_maps(inputs, sched, perB, perD):
    x = np.asarray(inputs["x"], np.float32).astype(np.float16)
    xpad = np.zeros((NPAD, DIN), np.float16)
    xpad[:N] = x
    W1 = np.asarray(inputs["W1"], np.float32)
    a_s1 = np.asarray(inputs["att_src1"], np.float32)
    a_d1 = np.asarray(inputs["att_dst1"], np.float32)
    W2 = np.asarray(inputs["W2"], np.float32)
    a_s2 = np.asarray(inputs["att_src2"], np.float32)[0]
    a_d2 = np.asarray(inputs["att_dst2"], np.float32)[0]
    assert not np.any(np.asarray(inputs["bias1"]))
    assert not np.any(np.asarray(inputs["bias2"]))

    perm1 = np.zeros(128, np.int64)
    for cc in range(C1):
        for h in range(H1):
            perm1[cc * H1 + h] = h * C1 + cc
    W1p = np.ascontiguousarray(W1[:, perm1])
    wa_src1 = np.zeros((DIN, H1), np.float32)
    wa_dst1 = np.zeros((DIN, H1), np.float32)
    for h in range(H1):
        wa_src1[:, h] = W1[:, h * C1:(h + 1) * C1] @ a_s1[h]
        wa_dst1[:, h] = W1[:, h * C1:(h + 1) * C1] @ a_d1[h]
    rhs1e = np.zeros((DIN, ROW1), np.float16)
    rhs1e[:, 0:128] = W1p
    rhs1e[:, 132:136] = wa_src1
    wad1 = wa_dst1.astype(np.float16)
    W2p = np.ascontiguousarray(W2[perm1, :])
    rhs2 = np.zeros((128, 34), np.float16)
    rhs2[:, 0:32] = W2p
    rhs2[:, 32] = W2p @ a_s2
    rhs2[:, 33] = W2p @ a_d2

    T1 = sched["T1"]
    in_maps = []
    for c in range(NCORE):
        srcm = perB[c]["xet_src"]                       # [128e, T1]
        valid = srcm >= 0
        g = xpad[np.where(valid, srcm, 0).reshape(-1)]  # [(128*T1), feat]
        g = g.reshape(128, T1, DIN)
        g[~valid] = 0
        xet = np.ascontiguousarray(
            np.transpose(g, (2, 1, 0))).reshape(DIN, T1 * 128)
        xl = xpad[c * CS:(c + 1) * CS].reshape(NMACRO, 128, DIN)
        xloc = np.ascontiguousarray(
            np.transpose(xl, (2, 0, 1))).reshape(DIN, NMACRO * 128)
        in_maps.append(dict(
            xet=xet, xloc=xloc, pat1=perB[c]["pat1"], patT1=perB[c]["patT1"],
            bdm=sched["bdm"], rhs1e=rhs1e, wad1=wad1, rhs2=rhs2,
            ilo=perD[c]["ilo"], ihi=perD[c]["ihi"],
            pat2=perD[c]["pat2"], patT2=perD[c]["patT2"],
        ))
    return in_maps


# ============================ bass program ==============================
def build_program(sched):
    T1, NST = sched["T1"], sched["NST"]
    schedB, stackB, groupsB = sched["schedB"], sched["stackB"], sched["groupsB"]
    nstack = sched["nstack"]
    GTB, GSB = sched["GTB"], sched["GSB"]
    groupsD, ntileD, koff = sched["groupsD"], sched["ntileD"], sched["koff"]
    tbase, T2 = sched["tbase"], sched["T2"]
    segW, segM = sched["segW"], sched["segM"]
    segW_g0, segM_g0 = sched["segW_g0"], sched["segM_g0"]
    NSW, NSM = sched["NSW"], sched["NSM"]
    glo_off, ghi_off = sched["glo_off"], sched["ghi_off"]
    GTD = max(ntileD)
    GSW = max(segW_g0[g + 1] - segW_g0[g] for g in range(len(groupsD)))
    GSM = max(segM_g0[g + 1] - segM_g0[g] for g in range(len(groupsD)))

    nc = bacc.Bacc("TRN2", target_bir_lowering=False, debug=False,
                   num_devices=NCORE)
    tc = tile.TileContext(nc)

    xet_d = nc.dram_tensor("xet", [128, T1 * 128], FP16, kind="ExternalInput")
    xloc_d = nc.dram_tensor("xloc", [128, NMACRO * 128], FP16, kind="ExternalInput")
    pat1_d = nc.dram_tensor("pat1", [128, T1 * WIN], FP16, kind="ExternalInput")
    patT1_d = nc.dram_tensor("patT1", [128, NST * 128], FP16, kind="ExternalInput")
    bdm_d = nc.dram_tensor("bdm", [128, 16], FP16, kind="ExternalInput")
    rhs1e_d = nc.dram_tensor("rhs1e", [128, ROW1], FP16, kind="ExternalInput")
    wad1_d = nc.dram_tensor("wad1", [128, 4], FP16, kind="ExternalInput")
    rhs2_d = nc.dram_tensor("rhs2", [128, 34], FP16, kind="ExternalInput")
    ilo_d = nc.dram_tensor("ilo", [128, sched["ilo_cols"]], I16, kind="ExternalInput")
    ihi_d = nc.dram_tensor("ihi", [128, sched["ihi_cols"]], I16, kind="ExternalInput")
    pat2_d = nc.dram_tensor("pat2", [128, NSW * WIN], FP16, kind="ExternalInput")
    patT2_d = nc.dram_tensor("patT2", [128, NSM * 128], FP16, kind="ExternalInput")
    out2_d = nc.dram_tensor("out2", [CS, C2], FP32, kind="ExternalOutput")
    dbg_ht_d = nc.dram_tensor("dbg_ht", [128, CS], FP16, kind="ExternalOutput")
    dbg_da_d = nc.dram_tensor("dbg_da", [128, NMACRO * 4], FP32, kind="ExternalOutput")
    dbg_d2_d = nc.dram_tensor("dbg_d2", [128, NMACRO], FP16, kind="ExternalOutput")
    dbg_g2_d = nc.dram_tensor("dbg_g2", [128, 8 * ROW2], FP16, kind="ExternalOutput")
    dbg_ed2_d = nc.dram_tensor("dbg_ed2", [128, 48], FP32, kind="ExternalOutput")
    tb2l_d = nc.dram_tensor("tb2l", [CS, ROW2], FP16)
    tb2g_d = nc.dram_tensor("tb2g", [NPAD, ROW2], FP16, addr_space="Shared")

    with tc, ExitStack() as ctx:
        cc_sem = ctx.enter_context(nc.semaphore("ccsem"))
        const = ctx.enter_context(tc.tile_pool(name="const", bufs=1))
        rhs1e_sb = const.tile([128, ROW1], FP16)
        nc.sync.dma_start(rhs1e_sb[:], rhs1e_d.ap())
        wad1_sb = const.tile([128, 4], FP16)
        nc.sync.dma_start(wad1_sb[:], wad1_d.ap())
        rhs2_sb = const.tile([128, 34], FP16)
        nc.sync.dma_start(rhs2_sb[:], rhs2_d.ap())
        bdm_sb = const.tile([128, 16], FP16)
        nc.sync.dma_start(bdm_sb[:], bdm_d.ap())
        ident16 = const.tile([128, 128], FP16)
        make_identity(nc, ident16[:])
        nsh1 = const.tile([128, 1], FP32)
        nc.gpsimd.memset(nsh1[:], -SHIFT1)
        nsh2 = const.tile([128, 1], FP32)
        nc.gpsimd.memset(nsh2[:], -SHIFT2)
        d_all = const.tile([128, NMACRO * 4], FP32)
        d2_all = const.tile([128, NMACRO], FP16)
        ht = const.tile([128, CS], FP16)

        # ---------------- phase A: local dst-dot table d_all ----------------
        with tc.tile_pool(name="xl", bufs=2) as xlp, \
             tc.tile_pool(name="pa", bufs=2, space="PSUM") as paps:
            CH = 8
            for m0 in range(0, NMACRO, CH):
                nch = min(CH, NMACRO - m0)
                xl = xlp.tile([128, CH * 128], FP16, tag="xl")
                nc.sync.dma_start(xl[:, 0:nch * 128],
                                  xloc_d.ap()[:, m0 * 128:(m0 + nch) * 128])
                dps = paps.tile([128, CH * 4], FP32, space="PSUM", tag="dps")
                for k in range(nch):
                    nc.tensor.matmul(dps[:, k * 4:(k + 1) * 4],
                                     xl[:, k * 128:(k + 1) * 128],
                                     wad1_sb[:], start=True, stop=True)
                nc.vector.tensor_copy(d_all[:, m0 * 4:(m0 + nch) * 4],
                                      dps[:, 0:nch * 4])

        # ---------------- phase B: layer 1 ----------------
        with tc.tile_pool(name="xe", bufs=2) as xep, \
             tc.tile_pool(name="pt1", bufs=2) as pt1p, \
             tc.tile_pool(name="ptt", bufs=2) as pttp, \
             tc.tile_pool(name="stg", bufs=2) as stgp, \
             tc.tile_pool(name="msgp", bufs=2) as msgp, \
             tc.tile_pool(name="zu", bufs=2) as zup, \
             tc.tile_pool(name="hps", bufs=2, space="PSUM") as hpsp, \
             tc.tile_pool(name="edps", bufs=2, space="PSUM") as edpsp, \
             tc.tile_pool(name="aggps", bufs=2, space="PSUM") as aggpsp, \
             tc.tile_pool(name="htps", bufs=2, space="PSUM") as htpsp, \
             tc.tile_pool(name="hn", bufs=2) as hnp:
            for (t0, t1, m0, m1, s0, s1) in groupsB:
                gt, gs = t1 - t0, s1 - s0
                xe = xep.tile([128, GTB * 128], FP16, tag="xe")
                nc.sync.dma_start(xe[:, 0:gt * 128],
                                  xet_d.ap()[:, t0 * 128:t1 * 128])
                pat = pt1p.tile([128, GTB * WIN], FP16, tag="pat")
                nc.sync.dma_start(pat[:, 0:gt * WIN],
                                  pat1_d.ap()[:, t0 * WIN:t1 * WIN])
                ptt = pttp.tile([128, GSB * 128], FP16, tag="ptt")
                nc.sync.dma_start(ptt[:, 0:gs * 128],
                                  patT1_d.ap()[:, s0 * 128:s1 * 128])
                stg = stgp.tile([128, GTB * ROW1], FP16, tag="stg")
                for j in range(gt):
                    hps = hpsp.tile([128, ROW1], FP32, space="PSUM", tag="hps")
                    nc.tensor.matmul(hps[:], xe[:, j * 128:(j + 1) * 128],
                                     rhs1e_sb[:], start=True, stop=True)
                    nc.scalar.activation(stg[:, j * ROW1:(j + 1) * ROW1], hps[:],
                                         mybir.ActivationFunctionType.Copy)
                stgv = stg[:].rearrange("p (t c) -> p t c", c=ROW1)
                nc.vector.memset(stgv[:, 0:gt, 128:132], 1.0)
                # dst dots via stacks
                edps = edpsp.tile([128, GSB * 16], FP32, space="PSUM", tag="ed")
                edr = zup.tile([128, GSB * 16], FP16, tag="edr")
                for si in range(s0, s1):
                    mac, s, _ = stackB[si]
                    sl = si - s0
                    nc.vector.tensor_tensor(
                        out=edr[:, sl * 16:(sl + 1) * 16]
                            .rearrange("p (w h) -> p w h", h=4),
                        in0=d_all[:, 4 * mac:4 * mac + 4].unsqueeze(1)
                            .broadcast_to([128, 4, 4]),
                        in1=bdm_sb[:].rearrange("p (w h) -> p w h", h=4),
                        op=mybir.AluOpType.mult)
                    nc.tensor.matmul(edps[:, sl * 16:(sl + 1) * 16],
                                     ptt[:, sl * 128:(sl + 1) * 128],
                                     edr[:, sl * 16:(sl + 1) * 16],
                                     start=True, stop=True)
                # z, u, msg
                z = zup.tile([128, GTB * 4], FP32, tag="z")
                nc.vector.tensor_tensor(
                    out=z[:, 0:gt * 4].rearrange("p (t h) -> p t h", h=4),
                    in0=stgv[:, 0:gt, 132:136],
                    in1=edps[:, 0:gt * 4].rearrange("p (t h) -> p t h", h=4),
                    op=mybir.AluOpType.add)
                zs = zup.tile([128, GTB * 4], FP32, tag="zs")
                nc.vector.tensor_scalar_mul(zs[:, 0:gt * 4], z[:, 0:gt * 4], 0.2)
                nc.vector.tensor_tensor(out=z[:, 0:gt * 4], in0=z[:, 0:gt * 4],
                                        in1=zs[:, 0:gt * 4],
                                        op=mybir.AluOpType.max)
                u = zup.tile([128, GTB * 4], FP16, tag="u")
                nc.scalar.activation(u[:, 0:gt * 4], z[:, 0:gt * 4],
                                     mybir.ActivationFunctionType.Exp,
                                     bias=nsh1[:])
                msg = msgp.tile([128, GTB * F1], FP16, tag="msg")
                nc.vector.tensor_tensor(
                    out=msg[:, 0:gt * F1]
                        .rearrange("p (t c h) -> p t c h", c=33, h=4),
                    in0=stgv[:, 0:gt, 0:132]
                        .rearrange("p t (c h) -> p t c h", h=4),
                    in1=u[:, 0:gt * 4].rearrange("p (t h) -> p t h", h=4)
                        .unsqueeze(2).broadcast_to([128, gt, 33, 4]),
                    op=mybir.AluOpType.mult)
                # aggregate per macro
                for mac in range(m0, m1):
                    agg = aggpsp.tile([128, F1], FP32, space="PSUM", tag="agg")
                    ns = nstack[mac]
                    tm0 = 4 * int(sum(nstack[:mac]))
                    for k in range(4 * ns):
                        t = tm0 + k
                        j = t - t0
                        _, w, s = schedB[t]
                        nc.tensor.matmul(
                            agg[w * WIN:(w + 1) * WIN, :],
                            pat[:, j * WIN:(j + 1) * WIN],
                            msg[:, j * F1:(j + 1) * F1],
                            start=(s == 0), stop=(s == ns - 1),
                            tile_position=(0, w * WIN))
                    r = hnp.tile([128, 4], FP32, tag="r")
                    nc.vector.reciprocal(r[:], agg[:, 128:132])
                    hn = hnp.tile([128, 128], FP16, tag="hn")
                    nc.vector.tensor_tensor(
                        out=hn[:].rearrange("p (c h) -> p c h", h=4),
                        in0=agg[:, 0:128].rearrange("p (c h) -> p c h", h=4),
                        in1=r[:].unsqueeze(1).broadcast_to([128, 32, 4]),
                        op=mybir.AluOpType.mult)
                    a = hnp.tile([128, 128], FP16, tag="elua")
                    nc.vector.tensor_scalar_min(a[:], hn[:], 0.0)
                    e = hnp.tile([128, 128], FP16, tag="elue")
                    nc.scalar.activation(e[:], a[:],
                                         mybir.ActivationFunctionType.Exp)
                    em1 = hnp.tile([128, 128], FP16, tag="eluem")
                    nc.vector.tensor_scalar_add(em1[:], e[:], -1.0)
                    helu = hnp.tile([128, 128], FP16, tag="helu")
                    nc.vector.tensor_tensor(out=helu[:], in0=hn[:], in1=em1[:],
                                            op=mybir.AluOpType.max)
                    htps = htpsp.tile([128, 128], FP16, space="PSUM", tag="htp")
                    nc.tensor.transpose(htps[:], helu[:], ident16[:])
                    nc.scalar.activation(ht[:, mac * 128:(mac + 1) * 128],
                                         htps[:],
                                         mybir.ActivationFunctionType.Copy)

        nc.sync.dma_start(dbg_ht_d.ap(), ht[:])
        nc.sync.dma_start(dbg_da_d.ap(), d_all[:])
        tc.strict_bb_all_engine_barrier()

        # ---------------- phase C: layer-2 node table ----------------
        with tc.tile_pool(name="pc_ps", bufs=4, space="PSUM") as pcps, \
             tc.tile_pool(name="pc_st", bufs=2) as pcst:
            CH2 = 8
            for m0 in range(0, NMACRO, CH2):
                nch = min(CH2, NMACRO - m0)
                st2 = pcst.tile([128, CH2 * ROW2], FP16, tag="st2")
                st2v = st2[:].rearrange("p (t c) -> p t c", c=ROW2)
                for k in range(nch):
                    mac = m0 + k
                    qps = pcps.tile([128, 34], FP32, space="PSUM", tag="q")
                    nc.tensor.matmul(qps[:], ht[:, mac * 128:(mac + 1) * 128],
                                     rhs2_sb[:], start=True, stop=True)
                    nc.scalar.activation(st2[:, k * ROW2:k * ROW2 + 32],
                                         qps[:, 0:32],
                                         mybir.ActivationFunctionType.Copy)
                    nc.vector.tensor_copy(st2v[:, k:k + 1, 33:34],
                                          qps[:, 32:33].unsqueeze(1))
                    nc.vector.tensor_copy(
                        d2_all[:, mac:mac + 1], qps[:, 33:34])
                nc.vector.memset(st2v[:, 0:nch, 32:33], 1.0)
                nc.vector.memset(st2v[:, 0:nch, 34:128], 0.0)
                dst2 = tb2l_d.ap().rearrange("(p t) c -> p t c", p=128)
                nc.sync.dma_start(dst2[:, m0:m0 + nch, :],
                                  st2v[:, 0:nch, :])

        tc.strict_bb_all_engine_barrier()
        with tc.tile_critical():
            nc.gpsimd.collective_compute(
                "AllGather", mybir.AluOpType.bypass,
                replica_groups=[list(range(NCORE))],
                ins=[tb2l_d.ap().opt()],
                outs=[tb2g_d.ap().opt()],
            ).then_inc(cc_sem)
            nc.gpsimd.wait_ge(cc_sem, 1)
        tc.strict_bb_all_engine_barrier()

        # ---------------- phase D: layer 2 ----------------
        ixlo = const.tile([128, sched["ilo_cols"]], I16)
        nc.sync.dma_start(ixlo[:], ilo_d.ap())
        ixhi = const.tile([128, sched["ihi_cols"]], I16)
        nc.sync.dma_start(ixhi[:], ihi_d.ap())
        tab_lo = tb2g_d.ap()[0:HA2, :]
        tab_hi = tb2g_d.ap()[HA2:NPAD, :]
        with tc.tile_pool(name="g2", bufs=2) as g2p, \
             tc.tile_pool(name="pt2", bufs=2) as pt2p, \
             tc.tile_pool(name="ptt2", bufs=2) as ptt2p, \
             tc.tile_pool(name="msg2", bufs=2) as msg2p, \
             tc.tile_pool(name="zu2", bufs=2) as zu2p, \
             tc.tile_pool(name="ed2ps", bufs=2, space="PSUM") as ed2psp, \
             tc.tile_pool(name="agg2ps", bufs=6, space="PSUM") as agg2psp, \
             tc.tile_pool(name="o2", bufs=2) as o2p:
            for g, (gm0, gm1) in enumerate(groupsD):
                nt = ntileD[g]
                ko = koff[g]
                t0 = int(tbase[g])
                sw0, sw1 = segW_g0[g], segW_g0[g + 1]
                sm0, sm1 = segM_g0[g], segM_g0[g + 1]
                g2 = g2p.tile([128, GTD * ROW2], FP16, tag="g2")
                g2v = g2[:].rearrange("p (t c) -> p t c", c=ROW2)
                nc.gpsimd.dma_gather(
                    g2v[:, 0:nt, :], tab_lo,
                    ixlo[:, glo_off[g]:glo_off[g] + nt * 8],
                    nt * 128, ko * 128, ROW2, single_packet=False)
                nc.gpsimd.dma_gather(
                    g2v[:, ko:nt, :], tab_hi,
                    ixhi[:, ghi_off[g]:ghi_off[g] + (nt - ko) * 8],
                    (nt - ko) * 128, (nt - ko) * 128, ROW2,
                    single_packet=False)
                pt2 = pt2p.tile([128, GSW * WIN], FP16, tag="pt2")
                nc.sync.dma_start(pt2[:, 0:(sw1 - sw0) * WIN],
                                  pat2_d.ap()[:, sw0 * WIN:sw1 * WIN])
                ptt2 = ptt2p.tile([128, GSM * 128], FP16, tag="ptt2")
                nc.sync.dma_start(ptt2[:, 0:(sm1 - sm0) * 128],
                                  patT2_d.ap()[:, sm0 * 128:sm1 * 128])
                if g == 0:
                    nc.sync.dma_start(dbg_g2_d.ap(), g2[:, 0:8 * ROW2])
                    nc.sync.dma_start(dbg_d2_d.ap(), d2_all[:])
                # dst dots
                ed2 = ed2psp.tile([128, GTD], FP32, space="PSUM", tag="ed2")
                for i in range(sm0, sm1):
                    t, mac, st_, sp_ = segM[i]
                    j = t - t0
                    nc.tensor.matmul(ed2[:, j:j + 1],
                                     ptt2[:, (i - sm0) * 128:(i - sm0 + 1) * 128],
                                     d2_all[:, mac:mac + 1],
                                     start=st_, stop=sp_)
                z2 = zu2p.tile([128, GTD], FP32, tag="z2")
                nc.vector.tensor_tensor(
                    out=z2[:, 0:nt].unsqueeze(2),
                    in0=g2v[:, 0:nt, 33:34],
                    in1=ed2[:, 0:nt].unsqueeze(2),
                    op=mybir.AluOpType.add)
                zs2 = zu2p.tile([128, GTD], FP32, tag="zs2")
                nc.vector.tensor_scalar_mul(zs2[:, 0:nt], z2[:, 0:nt], 0.2)
                nc.vector.tensor_tensor(out=z2[:, 0:nt], in0=z2[:, 0:nt],
                                        in1=zs2[:, 0:nt],
                                        op=mybir.AluOpType.max)
                if g == 0:
                    ed2dbg = zu2p.tile([128, 48], FP32, tag="ed2dbg")
                    nc.vector.tensor_copy(ed2dbg[:], ed2[:, 0:48])
                    nc.sync.dma_start(dbg_ed2_d.ap(), ed2dbg[:])
                u2 = zu2p.tile([128, GTD], FP16, tag="u2")
                nc.scalar.activation(u2[:, 0:nt], z2[:, 0:nt],
                                     mybir.ActivationFunctionType.Exp,
                                     bias=nsh2[:])
                msg2 = msg2p.tile([128, GTD * F2], FP16, tag="m2")
                nc.vector.tensor_tensor(
                    out=msg2[:, 0:nt * F2]
                        .rearrange("p (t c) -> p t c", c=F2),
                    in0=g2v[:, 0:nt, 0:33],
                    in1=u2[:, 0:nt].unsqueeze(2).broadcast_to([128, nt, 33]),
                    op=mybir.AluOpType.mult)
                # aggregate
                aggm = {}
                for mac in range(gm0, gm1):
                    aggm[mac] = agg2psp.tile([128, F2], FP32, space="PSUM",
                                             tag="ag2", name=f"ag2_{mac}")
                for i in range(sw0, sw1):
                    t, mac, w, st_, sp_ = segW[i]
                    j = t - t0
                    nc.tensor.matmul(
                        aggm[mac][w * WIN:(w + 1) * WIN, :],
                        pt2[:, (i - sw0) * WIN:(i - sw0 + 1) * WIN],
                        msg2[:, j * F2:(j + 1) * F2],
                        start=st_, stop=sp_, tile_position=(0, w * WIN))
                for mac in range(gm0, gm1):
                    r2 = o2p.tile([128, 1], FP32, tag="r2")
                    nc.vector.reciprocal(r2[:], aggm[mac][:, 32:33])
                    o2 = o2p.tile([128, C2], FP32, tag="o2")
                    nc.vector.tensor_tensor(
                        out=o2[:], in0=aggm[mac][:, 0:C2],
                        in1=r2[:].broadcast_to([128, C2]),
                        op=mybir.AluOpType.mult)
                    nc.sync.dma_start(
                        out2_d.ap()[mac * 128:(mac + 1) * 128, :], o2[:])

    nc.compile()
    return nc


_CACHE = {}


def run(inputs, trace=False):
    sched, perB, perD = host_prep(inputs)
    in_maps = make_in_maps(inputs, sched, perB, perD)
    key = (sched["T1"], sched["T2"], sched["NSW"], sched["NSM"])
    if key not in _CACHE:
        _CACHE[key] = build_program(sched)
    nc = _CACHE[key]
    res = run_bass_kernel_spmd(nc, in_maps, core_ids=list(range(NCORE)),
                               trace=trace)
    out = np.zeros((N, C2), np.float32)
    for c in range(NCORE):
        lo = c * CS
        hi = min(lo + CS, N)
        out[lo:hi] = res.results[c]["out2"][: hi - lo]
    return out, res


def kernel(**inputs):
    out, _res = run(inputs)
    return out.astype(np.float32)


# revision 12
# speedup vs baseline: 3.8489x; 1.4386x over previous
# 2-layer GAT on trn2 x8 — dst-partitioned (graph parallel).
#
# Layer 1 avoids all device-side gathers: the host pre-gathers x rows into
# dst-sorted edge-slot order (pure layout prep), and the device projects
# per-edge features with PE matmuls (h | att-src dots in one pass). Per-edge
# dst-attention dots come from stacked one-hot transpose matmuls against the
# per-node dot table. Aggregation (numerator + softmax denominator) is the
# baseline's one-hot window matmul.
#
# Layer 2 gathers the AllGather'd per-node table (256B rows: h2 | 1 | u2)
# with batched gpsimd dma_gather (2 calls per 3-macro group, lo/hi table
# halves since gather indices are int16), replacing the baseline's ~4000
# per-tile INDIRECT1D descents (~1.1us fixed cost each).
#
# This file is self-contained: it inlines host schedule construction
# (host_prep/host_blobs equivalents) and hardcodes all shapes.
from contextlib import ExitStack

import numpy as np

import concourse.bass as bass
import concourse.bacc as bacc
import concourse.tile as tile
from concourse import mybir
from concourse.bass_utils import run_bass_kernel_spmd
from concourse.masks import make_identity

# ---- problem constants ----
N = 50000
DIN = 128
H1, C1 = 4, 32
C2 = 32
NCORE = 8
CS = 6272
NMACRO = CS // 128        # 49
WIN = 32
NW = 4
EPT = 128
GM_B = 2                  # phase-B group: macros
GM_D = 3                  # phase-D group: macros
HA2 = 25088               # tab2 lo half (cores 0-3)
ROW1 = 136                # proj psum/staging row: h(128) | ones(4) | u_src(4)
F1 = 132
F2 = 33
ROW2 = 128                # tab2 row: h2(32) | 1.0 | u2 | pad
SHIFT1, SHIFT2 = 6.0, 2.0

FP16 = mybir.dt.float16
FP32 = mybir.dt.float32
I16 = mybir.dt.int16

NPAD = NCORE * CS         # 50176


def _phys2(n):
    core, slot = divmod(int(n), CS)
    return core * CS + (slot % 128) * NMACRO + slot // 128


def _wrap16(a):
    a = np.asarray(a, np.int64)
    n = len(a)
    cols = max(1, (n + 15) // 16)
    out = np.full((16, cols), -1, np.int16)
    out[np.arange(n) % 16, np.arange(n) // 16] = a.astype(np.int16)
    return np.tile(out, (8, 1))


# ======================== host schedule + blobs =========================
def host_prep(inputs):
    ei = np.asarray(inputs["edge_index"])
    # self-loops for all nodes incl. pads (keeps every dst row non-empty so
    # no inf/nan reciprocals leak through the dst-dot matmuls)
    src_all = np.concatenate([ei[0].astype(np.int64),
                              np.arange(NPAD, dtype=np.int64)])
    dst_all = np.concatenate([ei[1].astype(np.int64),
                              np.arange(NPAD, dtype=np.int64)])

    cores = []
    for c in range(NCORE):
        base = c * CS
        m = (dst_all >= base) & (dst_all < base + CS)
        s_c = src_all[m]
        r_c = dst_all[m] - base
        order = np.argsort(r_c, kind="stable")
        s_c, r_c = s_c[order], r_c[order]
        deg = np.bincount(r_c, minlength=CS)
        assert deg.max() <= EPT
        cores.append(dict(s=s_c, r=r_c, deg=deg))

    # ---- phase B schedule: window-pure tiles, padded to full stacks ----
    tpw = np.zeros((NMACRO, NW), np.int64)
    core_wt = []
    for c in range(NCORE):
        deg = cores[c]["deg"]
        wt = {}
        for mac in range(NMACRO):
            for w in range(NW):
                lo = mac * 128 + w * WIN
                tl, cur, cur_e = [], [], 0
                for r in range(lo, lo + WIN):
                    dd = int(deg[r])
                    if dd == 0:
                        continue
                    if cur_e + dd > EPT:
                        tl.append(cur)
                        cur, cur_e = [], 0
                    cur.append(r)
                    cur_e += dd
                if cur:
                    tl.append(cur)
                wt[(mac, w)] = tl
                tpw[mac, w] = max(tpw[mac, w], len(tl))
        core_wt.append(wt)

    schedB, nstack, stackB = [], [], []
    for mac in range(NMACRO):
        ns = int(tpw[mac].max())
        nstack.append(ns)
        for s in range(ns):
            stackB.append((mac, s, len(schedB)))
            for w in range(NW):
                schedB.append((mac, w, s))
    T1, NST = len(schedB), len(stackB)

    macB0 = np.zeros(NMACRO + 1, np.int64)
    for i, (mac, w, s) in enumerate(schedB):
        macB0[mac + 1] = i + 1
    st0 = np.zeros(NMACRO + 1, np.int64)
    for i, (mac, s, t0) in enumerate(stackB):
        st0[mac + 1] = i + 1
    groupsB = []
    for g0 in range(0, NMACRO, GM_B):
        g1 = min(g0 + GM_B, NMACRO)
        groupsB.append((int(macB0[g0]), int(macB0[g1]), g0, g1,
                       int(st0[g0]), int(st0[g1])))
    GTB = max(t1 - t0 for t0, t1, *_ in groupsB)
    GSB = max(s1 - s0 for *_, s0, s1 in groupsB)

    perB = []
    for c in range(NCORE):
        s_c, deg = cores[c]["s"], cores[c]["deg"]
        starts = np.zeros(CS + 1, np.int64)
        np.cumsum(deg, out=starts[1:])
        wt = core_wt[c]
        xet_src = np.full((128, T1), -1, np.int64)
        pat1 = np.zeros((128, T1 * WIN), np.float16)
        for t, (mac, w, s) in enumerate(schedB):
            tl = wt[(mac, w)]
            if s >= len(tl):
                continue
            e = 0
            for r in tl[s]:
                for k in range(starts[r], starts[r + 1]):
                    xet_src[e, t] = s_c[k]
                    pat1[e, t * WIN + (r - mac * 128 - w * WIN)] = 1.0
                    e += 1
        patT1 = np.zeros((128, NST * 128), np.float16)
        for i, (mac, s, t0) in enumerate(stackB):
            for w in range(NW):
                blk = pat1[:, (t0 + w) * WIN:(t0 + w + 1) * WIN]
                patT1[32 * w:32 * (w + 1), i * 128:(i + 1) * 128] = blk.T
        perB.append(dict(xet_src=xet_src, pat1=pat1, patT1=patT1))

    bdm = np.zeros((128, 16), np.float16)
    for w in range(NW):
        bdm[32 * w:32 * (w + 1), 4 * w:4 * w + 4] = 1.0

    # ---- phase D schedule: dense tiles, lo/hi per group ----
    groupsD = [(g0, min(g0 + GM_D, NMACRO)) for g0 in range(0, NMACRO, GM_D)]
    ND = len(groupsD)
    core_lohi = []
    for c in range(NCORE):
        s_c, r_c = cores[c]["s"], cores[c]["r"]
        ent = []
        for (g0, g1) in groupsD:
            m = (r_c >= g0 * 128) & (r_c < g1 * 128)
            sg, rg = s_c[m], r_c[m]
            is_lo = np.array([_phys2(s) < HA2 for s in sg], bool) \
                if len(sg) else np.zeros(0, bool)
            ent.append((sg[is_lo], rg[is_lo], sg[~is_lo], rg[~is_lo]))
        core_lohi.append(ent)

    nloM = [max(len(core_lohi[c][g][0]) for c in range(NCORE)) for g in range(ND)]
    nhiM = [max(len(core_lohi[c][g][2]) for c in range(NCORE)) for g in range(ND)]
    koff = [(n + 127) // 128 for n in nloM]
    ntileD = [koff[g] + (nhiM[g] + 127) // 128 for g in range(ND)]
    T2 = int(np.sum(ntileD))
    tbase = np.zeros(ND + 1, np.int64)
    for g in range(ND):
        tbase[g + 1] = tbase[g] + ntileD[g]

    slot_r = np.full((NCORE, 128, T2), -1, np.int64)
    slot_s = np.full((NCORE, 128, T2), -1, np.int64)
    for c in range(NCORE):
        for g in range(ND):
            slo, rlo, shi, rhi = core_lohi[c][g]
            t0 = int(tbase[g])
            jj = np.arange(len(slo))
            slot_r[c, jj % 128, t0 + jj // 128] = rlo
            slot_s[c, jj % 128, t0 + jj // 128] = slo
            hb_ = t0 + koff[g]
            jj = np.arange(len(shi))
            slot_r[c, jj % 128, hb_ + jj // 128] = rhi
            slot_s[c, jj % 128, hb_ + jj // 128] = shi

    segW, segM = [], []
    segW_g0, segM_g0 = [0], [0]
    for g in range(ND):
        t0, t1 = int(tbase[g]), int(tbase[g + 1])
        winset, macset = {}, {}
        for t in range(t0, t1):
            rr = slot_r[:, :, t]
            rr = rr[rr >= 0]
            if len(rr) == 0:
                continue
            for wv in sorted(set(int(r) // WIN for r in rr)):
                winset.setdefault(wv, []).append(t)
            for mv in sorted(set(int(r) // 128 for r in rr)):
                macset.setdefault(mv, []).append(t)
        for wv, ts in sorted(winset.items()):
            for i, t in enumerate(ts):
                segW.append((t, wv // NW, wv % NW, i == 0, i == len(ts) - 1))
        # ed2 accumulates per TILE across its macros: flags per tile.
        # Fully-pad tiles get a dummy seg (zero patT2) so their ed2 psum
        # column is written (avoids stale-psum inf/nan reaching exp()).
        for t in range(t0, t1):
            rr = slot_r[:, :, t]
            rr = rr[rr >= 0]
            ms = sorted(set(int(r) // 128 for r in rr)) \
                if len(rr) else [groupsD[g][0]]
            for k, mv in enumerate(ms):
                segM.append((t, mv, k == 0, k == len(ms) - 1))
        segW_g0.append(len(segW))
        segM_g0.append(len(segM))
    NSW, NSM = len(segW), len(segM)

    perD = []
    glo_off, ghi_off = [], []
    for c in range(NCORE):
        pat2 = np.zeros((128, NSW * WIN), np.float16)
        patT2 = np.zeros((128, NSM * 128), np.float16)
        for i, (t, mac, w, st, sp) in enumerate(segW):
            rr = slot_r[c, :, t]
            sel = (rr >= 0) & (rr // WIN == mac * NW + w)
            for p in np.nonzero(sel)[0]:
                pat2[p, i * WIN + int(rr[p]) - (mac * NW + w) * WIN] = 1.0
        for i, (t, mac, st, sp) in enumerate(segM):
            rr = slot_r[c, :, t]
            sel = (rr >= 0) & (rr // 128 == mac)
            for p in np.nonzero(sel)[0]:
                patT2[int(rr[p]) - mac * 128, i * 128 + p] = 1.0
        ilo_cols = sum(koff[g] * 8 for g in range(ND))
        ihi_cols = sum((ntileD[g] - koff[g]) * 8 for g in range(ND))
        ilo = np.full((128, ilo_cols), -1, np.int16)
        ihi = np.zeros((128, ihi_cols), np.int16)
        olo = ohi = 0
        glo_off, ghi_off = [], []
        for g in range(ND):
            slo = core_lohi[c][g][0]
            shi = core_lohi[c][g][2]
            nlo_call = koff[g] * 128
            nhi_call = (ntileD[g] - koff[g]) * 128
            a = np.zeros(nlo_call, np.int64)
            if len(slo):
                a[: len(slo)] = [_phys2(s) for s in slo]
            b = np.zeros(nhi_call, np.int64)
            if len(shi):
                b[: len(shi)] = [_phys2(s) - HA2 for s in shi]
            ilo[:, olo: olo + nlo_call // 16] = _wrap16(a)
            ihi[:, ohi: ohi + nhi_call // 16] = _wrap16(b)
            glo_off.append(olo)
            ghi_off.append(ohi)
            olo += nlo_call // 16
            ohi += nhi_call // 16
        perD.append(dict(pat2=pat2, patT2=patT2, ilo=ilo, ihi=ihi))

    return dict(
        schedB=schedB, stackB=stackB, groupsB=groupsB, T1=T1, NST=NST,
        GTB=GTB, GSB=GSB, nstack=nstack, bdm=bdm,
        groupsD=groupsD, ntileD=ntileD, koff=koff, tbase=tbase, T2=T2,
        segW=segW, segM=segM, NSW=NSW, NSM=NSM,
        segW_g0=segW_g0, segM_g0=segM_g0,
        glo_off=glo_off, ghi_off=ghi_off,
        ilo_cols=ilo_cols, ihi_cols=ihi_cols,
    ), perB, perD


def make_in_maps(inputs, sched, perB, perD):
    x = np.asarray(inputs["x"], np.float32).astype(np.float16)
    xpad = np.zeros((NPAD, DIN), np.float16)
    xpad[:N] = x
    W1 = np.asarray(inputs["W1"], np.float32)
    a_s1 = np.asarray(inputs["att_src1"], np.float32)
    a_d1 = np.asarray(inputs["att_dst1"], np.float32)
    W2 = np.asarray(inputs["W2"], np.float32)
    a_s2 = np.asarray(inputs["att_src2"], np.float32)[0]
    a_d2 = np.asarray(inputs["att_dst2"], np.float32)[0]
    assert not np.any(np.asarray(inputs["bias1"]))
    assert not np.any(np.asarray(inputs["bias2"]))

    perm1 = np.zeros(128, np.int64)
    for cc in range(C1):
        for h in range(H1):
            perm1[cc * H1 + h] = h * C1 + cc
    W1p = np.ascontiguousarray(W1[:, perm1])
    wa_src1 = np.zeros((DIN, H1), np.float32)
    wa_dst1 = np.zeros((DIN, H1), np.float32)
    for h in range(H1):
        wa_src1[:, h] = W1[:, h * C1:(h + 1) * C1] @ a_s1[h]
        wa_dst1[:, h] = W1[:, h * C1:(h + 1) * C1] @ a_d1[h]
    rhs1e = np.zeros((DIN, ROW1), np.float16)
    rhs1e[:, 0:128] = W1p
    rhs1e[:, 132:136] = wa_src1
    wad1 = wa_dst1.astype(np.float16)
    W2p = np.ascontiguousarray(W2[perm1, :])
    rhs2 = np.zeros((128, 34), np.float16)
    rhs2[:, 0:32] = W2p
    rhs2[:, 32] = W2p @ a_s2
    rhs2[:, 33] = W2p @ a_d2

    T1 = sched["T1"]
    in_maps = []
    for c in range(NCORE):
        srcm = perB[c]["xet_src"]                       # [128e, T1]
        valid = srcm >= 0
        g = xpad[np.where(valid, srcm, 0).reshape(-1)]  # [(128*T1), feat]
        g = g.reshape(128, T1, DIN)
        g[~valid] = 0
        xet = np.ascontiguousarray(
            np.transpose(g, (2, 1, 0))).reshape(DIN, T1 * 128)
        xl = xpad[c * CS:(c + 1) * CS].reshape(NMACRO, 128, DIN)
        xloc = np.ascontiguousarray(
            np.transpose(xl, (2, 0, 1))).reshape(DIN, NMACRO * 128)
        in_maps.append(dict(
            xet=xet, xloc=xloc, pat1=perB[c]["pat1"], patT1=perB[c]["patT1"],
            bdm=sched["bdm"], rhs1e=rhs1e, wad1=wad1, rhs2=rhs2,
            ilo=perD[c]["ilo"], ihi=perD[c]["ihi"],
            pat2=perD[c]["pat2"], patT2=perD[c]["patT2"],
        ))
    return in_maps


# ============================ bass program ==============================
def build_program(sched):
    T1, NST = sched["T1"], sched["NST"]
    schedB, stackB, groupsB = sched["schedB"], sched["stackB"], sched["groupsB"]
    nstack = sched["nstack"]
    GTB, GSB = sched["GTB"], sched["GSB"]
    groupsD, ntileD, koff = sched["groupsD"], sched["ntileD"], sched["koff"]
    tbase, T2 = sched["tbase"], sched["T2"]
    segW, segM = sched["segW"], sched["segM"]
    segW_g0, segM_g0 = sched["segW_g0"], sched["segM_g0"]
    NSW, NSM = sched["NSW"], sched["NSM"]
    glo_off, ghi_off = sched["glo_off"], sched["ghi_off"]
    GTD = max(ntileD)
    GSW = max(segW_g0[g + 1] - segW_g0[g] for g in range(len(groupsD)))
    GSM = max(segM_g0[g + 1] - segM_g0[g] for g in range(len(groupsD)))

    nc = bacc.Bacc("TRN2", target_bir_lowering=False, debug=False,
                   num_devices=NCORE, dynamic_dma_scratch_size=32768)
    tc = tile.TileContext(nc)

    xet_d = nc.dram_tensor("xet", [128, T1 * 128], FP16, kind="ExternalInput")
    xloc_d = nc.dram_tensor("xloc", [128, NMACRO * 128], FP16, kind="ExternalInput")
    pat1_d = nc.dram_tensor("pat1", [128, T1 * WIN], FP16, kind="ExternalInput")
    patT1_d = nc.dram_tensor("patT1", [128, NST * 128], FP16, kind="ExternalInput")
    bdm_d = nc.dram_tensor("bdm", [128, 16], FP16, kind="ExternalInput")
    rhs1e_d = nc.dram_tensor("rhs1e", [128, ROW1], FP16, kind="ExternalInput")
    wad1_d = nc.dram_tensor("wad1", [128, 4], FP16, kind="ExternalInput")
    rhs2_d = nc.dram_tensor("rhs2", [128, 34], FP16, kind="ExternalInput")
    ilo_d = nc.dram_tensor("ilo", [128, sched["ilo_cols"]], I16, kind="ExternalInput")
    ihi_d = nc.dram_tensor("ihi", [128, sched["ihi_cols"]], I16, kind="ExternalInput")
    pat2_d = nc.dram_tensor("pat2", [128, NSW * WIN], FP16, kind="ExternalInput")
    patT2_d = nc.dram_tensor("patT2", [128, NSM * 128], FP16, kind="ExternalInput")
    out2_d = nc.dram_tensor("out2", [CS, C2], FP32, kind="ExternalOutput")
    dbg_ht_d = nc.dram_tensor("dbg_ht", [128, CS], FP16, kind="ExternalOutput")
    dbg_da_d = nc.dram_tensor("dbg_da", [128, NMACRO * 4], FP32, kind="ExternalOutput")
    dbg_d2_d = nc.dram_tensor("dbg_d2", [128, NMACRO], FP16, kind="ExternalOutput")
    dbg_g2_d = nc.dram_tensor("dbg_g2", [128, 8 * ROW2], FP16, kind="ExternalOutput")
    dbg_ed2_d = nc.dram_tensor("dbg_ed2", [128, 48], FP32, kind="ExternalOutput")
    tb2l_d = nc.dram_tensor("tb2l", [CS, ROW2], FP16)
    tb2g_d = nc.dram_tensor("tb2g", [NPAD, ROW2], FP16, addr_space="Shared")

    with tc, ExitStack() as ctx:
        cc_sem = ctx.enter_context(nc.semaphore("ccsem"))
        const = ctx.enter_context(tc.tile_pool(name="const", bufs=1))
        rhs1e_sb = const.tile([128, ROW1], FP16)
        nc.sync.dma_start(rhs1e_sb[:], rhs1e_d.ap())
        wad1_sb = const.tile([128, 4], FP16)
        nc.sync.dma_start(wad1_sb[:], wad1_d.ap())
        rhs2_sb = const.tile([128, 34], FP16)
        nc.sync.dma_start(rhs2_sb[:], rhs2_d.ap())
        bdm_sb = const.tile([128, 16], FP16)
        nc.sync.dma_start(bdm_sb[:], bdm_d.ap())
        ident16 = const.tile([128, 128], FP16)
        make_identity(nc, ident16[:])
        nsh1 = const.tile([128, 1], FP32)
        nc.gpsimd.memset(nsh1[:], -SHIFT1)
        nsh2 = const.tile([128, 1], FP32)
        nc.gpsimd.memset(nsh2[:], -SHIFT2)
        d_all = const.tile([128, NMACRO * 4], FP32)
        d2_all = const.tile([128, NMACRO], FP16)
        ht = const.tile([128, CS], FP16)

        # ---------------- phase A: local dst-dot table d_all ----------------
        with tc.tile_pool(name="xl", bufs=2) as xlp, \
             tc.tile_pool(name="pa", bufs=2, space="PSUM") as paps:
            CH = 8
            for m0 in range(0, NMACRO, CH):
                nch = min(CH, NMACRO - m0)
                xl = xlp.tile([128, CH * 128], FP16, tag="xl")
                nc.sync.dma_start(xl[:, 0:nch * 128],
                                  xloc_d.ap()[:, m0 * 128:(m0 + nch) * 128])
                dps = paps.tile([128, CH * 4], FP32, space="PSUM", tag="dps")
                for k in range(nch):
                    nc.tensor.matmul(dps[:, k * 4:(k + 1) * 4],
                                     xl[:, k * 128:(k + 1) * 128],
                                     wad1_sb[:], start=True, stop=True)
                nc.vector.tensor_copy(d_all[:, m0 * 4:(m0 + nch) * 4],
                                      dps[:, 0:nch * 4])

        # ---------------- phase B: layer 1 ----------------
        with tc.tile_pool(name="xe", bufs=2) as xep, \
             tc.tile_pool(name="pt1", bufs=2) as pt1p, \
             tc.tile_pool(name="ptt", bufs=2) as pttp, \
             tc.tile_pool(name="stg", bufs=2) as stgp, \
             tc.tile_pool(name="msgp", bufs=2) as msgp, \
             tc.tile_pool(name="zu", bufs=2) as zup, \
             tc.tile_pool(name="hps", bufs=2, space="PSUM") as hpsp, \
             tc.tile_pool(name="edps", bufs=2, space="PSUM") as edpsp, \
             tc.tile_pool(name="aggps", bufs=2, space="PSUM") as aggpsp, \
             tc.tile_pool(name="htps", bufs=2, space="PSUM") as htpsp, \
             tc.tile_pool(name="hn", bufs=2) as hnp:
            for (t0, t1, m0, m1, s0, s1) in groupsB:
                gt, gs = t1 - t0, s1 - s0
                xe = xep.tile([128, GTB * 128], FP16, tag="xe")
                nc.sync.dma_start(xe[:, 0:gt * 128],
                                  xet_d.ap()[:, t0 * 128:t1 * 128])
                pat = pt1p.tile([128, GTB * WIN], FP16, tag="pat")
                nc.sync.dma_start(pat[:, 0:gt * WIN],
                                  pat1_d.ap()[:, t0 * WIN:t1 * WIN])
                ptt = pttp.tile([128, GSB * 128], FP16, tag="ptt")
                nc.sync.dma_start(ptt[:, 0:gs * 128],
                                  patT1_d.ap()[:, s0 * 128:s1 * 128])
                stg = stgp.tile([128, GTB * ROW1], FP16, tag="stg")
                for j in range(gt):
                    hps = hpsp.tile([128, ROW1], FP32, space="PSUM", tag="hps")
                    nc.tensor.matmul(hps[:], xe[:, j * 128:(j + 1) * 128],
                                     rhs1e_sb[:], start=True, stop=True)
                    nc.scalar.activation(stg[:, j * ROW1:(j + 1) * ROW1], hps[:],
                                         mybir.ActivationFunctionType.Copy)
                stgv = stg[:].rearrange("p (t c) -> p t c", c=ROW1)
                nc.vector.memset(stgv[:, 0:gt, 128:132], 1.0)
                # dst dots via stacks
                edps = edpsp.tile([128, GSB * 16], FP32, space="PSUM", tag="ed")
                edr = zup.tile([128, GSB * 16], FP16, tag="edr")
                for si in range(s0, s1):
                    mac, s, _ = stackB[si]
                    sl = si - s0
                    nc.vector.tensor_tensor(
                        out=edr[:, sl * 16:(sl + 1) * 16]
                            .rearrange("p (w h) -> p w h", h=4),
                        in0=d_all[:, 4 * mac:4 * mac + 4].unsqueeze(1)
                            .broadcast_to([128, 4, 4]),
                        in1=bdm_sb[:].rearrange("p (w h) -> p w h", h=4),
                        op=mybir.AluOpType.mult)
                    nc.tensor.matmul(edps[:, sl * 16:(sl + 1) * 16],
                                     ptt[:, sl * 128:(sl + 1) * 128],
                                     edr[:, sl * 16:(sl + 1) * 16],
                                     start=True, stop=True)
                # z, u, msg
                z = zup.tile([128, GTB * 4], FP32, tag="z")
                nc.vector.tensor_tensor(
                    out=z[:, 0:gt * 4].rearrange("p (t h) -> p t h", h=4),
                    in0=stgv[:, 0:gt, 132:136],
                    in1=edps[:, 0:gt * 4].rearrange("p (t h) -> p t h", h=4),
                    op=mybir.AluOpType.add)
                zs = zup.tile([128, GTB * 4], FP32, tag="zs")
                nc.vector.tensor_scalar_mul(zs[:, 0:gt * 4], z[:, 0:gt * 4], 0.2)
                nc.vector.tensor_tensor(out=z[:, 0:gt * 4], in0=z[:, 0:gt * 4],
                                        in1=zs[:, 0:gt * 4],
                                        op=mybir.AluOpType.max)
                u = zup.tile([128, GTB * 4], FP16, tag="u")
                nc.scalar.activation(u[:, 0:gt * 4], z[:, 0:gt * 4],
                                     mybir.ActivationFunctionType.Exp,
                                     bias=nsh1[:])
                msg = msgp.tile([128, GTB * F1], FP16, tag="msg")
                nc.vector.tensor_tensor(
                    out=msg[:, 0:gt * F1]
                        .rearrange("p (t c h) -> p t c h", c=33, h=4),
                    in0=stgv[:, 0:gt, 0:132]
                        .rearrange("p t (c h) -> p t c h", h=4),
                    in1=u[:, 0:gt * 4].rearrange("p (t h) -> p t h", h=4)
                        .unsqueeze(2).broadcast_to([128, gt, 33, 4]),
                    op=mybir.AluOpType.mult)
                # aggregate per macro
                for mac in range(m0, m1):
                    agg = aggpsp.tile([128, F1], FP32, space="PSUM", tag="agg")
                    ns = nstack[mac]
                    tm0 = 4 * int(sum(nstack[:mac]))
                    for k in range(4 * ns):
                        t = tm0 + k
                        j = t - t0
                        _, w, s = schedB[t]
                        nc.tensor.matmul(
                            agg[w * WIN:(w + 1) * WIN, :],
                            pat[:, j * WIN:(j + 1) * WIN],
                            msg[:, j * F1:(j + 1) * F1],
                            start=(s == 0), stop=(s == ns - 1),
                            tile_position=(0, w * WIN))
                    r = hnp.tile([128, 4], FP32, tag="r")
                    nc.vector.reciprocal(r[:], agg[:, 128:132])
                    hn = hnp.tile([128, 128], FP16, tag="hn")
                    nc.vector.tensor_tensor(
                        out=hn[:].rearrange("p (c h) -> p c h", h=4),
                        in0=agg[:, 0:128].rearrange("p (c h) -> p c h", h=4),
                        in1=r[:].unsqueeze(1).broadcast_to([128, 32, 4]),
                        op=mybir.AluOpType.mult)
                    a = hnp.tile([128, 128], FP16, tag="elua")
                    nc.vector.tensor_scalar_min(a[:], hn[:], 0.0)
                    e = hnp.tile([128, 128], FP16, tag="elue")
                    nc.scalar.activation(e[:], a[:],
                                         mybir.ActivationFunctionType.Exp)
                    em1 = hnp.tile([128, 128], FP16, tag="eluem")
                    nc.vector.tensor_scalar_add(em1[:], e[:], -1.0)
                    helu = hnp.tile([128, 128], FP16, tag="helu")
                    nc.vector.tensor_tensor(out=helu[:], in0=hn[:], in1=em1[:],
                                            op=mybir.AluOpType.max)
                    htps = htpsp.tile([128, 128], FP16, space="PSUM", tag="htp")
                    nc.tensor.transpose(htps[:], helu[:], ident16[:])
                    nc.scalar.activation(ht[:, mac * 128:(mac + 1) * 128],
                                         htps[:],
                                         mybir.ActivationFunctionType.Copy)

        nc.sync.dma_start(dbg_ht_d.ap(), ht[:])
        nc.sync.dma_start(dbg_da_d.ap(), d_all[:])
        tc.strict_bb_all_engine_barrier()

        # ---------------- phase C: layer-2 node table ----------------
        with tc.tile_pool(name="pc_ps", bufs=4, space="PSUM") as pcps, \
             tc.tile_pool(name="pc_st", bufs=2) as pcst:
            CH2 = 8
            for m0 in range(0, NMACRO, CH2):
                nch = min(CH2, NMACRO - m0)
                st2 = pcst.tile([128, CH2 * ROW2], FP16, tag="st2")
                st2v = st2[:].rearrange("p (t c) -> p t c", c=ROW2)
                for k in range(nch):
                    mac = m0 + k
                    qps = pcps.tile([128, 34], FP32, space="PSUM", tag="q")
                    nc.tensor.matmul(qps[:], ht[:, mac * 128:(mac + 1) * 128],
                                     rhs2_sb[:], start=True, stop=True)
                    nc.scalar.activation(st2[:, k * ROW2:k * ROW2 + 32],
                                         qps[:, 0:32],
                                         mybir.ActivationFunctionType.Copy)
                    nc.vector.tensor_copy(st2v[:, k:k + 1, 33:34],
                                          qps[:, 32:33].unsqueeze(1))
                    nc.vector.tensor_copy(
                        d2_all[:, mac:mac + 1], qps[:, 33:34])
                nc.vector.memset(st2v[:, 0:nch, 32:33], 1.0)
                nc.vector.memset(st2v[:, 0:nch, 34:128], 0.0)
                dst2 = tb2l_d.ap().rearrange("(p t) c -> p t c", p=128)
                nc.sync.dma_start(dst2[:, m0:m0 + nch, :],
                                  st2v[:, 0:nch, :])

        tc.strict_bb_all_engine_barrier()
        with tc.tile_critical():
            nc.gpsimd.collective_compute(
                "AllGather", mybir.AluOpType.bypass,
                replica_groups=[list(range(NCORE))],
                ins=[tb2l_d.ap().opt()],
                outs=[tb2g_d.ap().opt()],
            ).then_inc(cc_sem)
            nc.gpsimd.wait_ge(cc_sem, 1)
        tc.strict_bb_all_engine_barrier()

        # ---------------- phase D: layer 2 ----------------
        ixlo = const.tile([128, sched["ilo_cols"]], I16)
        nc.sync.dma_start(ixlo[:], ilo_d.ap())
        ixhi = const.tile([128, sched["ihi_cols"]], I16)
        nc.sync.dma_start(ixhi[:], ihi_d.ap())
        tab_lo = tb2g_d.ap()[0:HA2, :]
        tab_hi = tb2g_d.ap()[HA2:NPAD, :]
        with tc.tile_pool(name="g2", bufs=2) as g2p, \
             tc.tile_pool(name="pt2", bufs=2) as pt2p, \
             tc.tile_pool(name="ptt2", bufs=2) as ptt2p, \
             tc.tile_pool(name="msg2", bufs=2) as msg2p, \
             tc.tile_pool(name="zu2", bufs=2) as zu2p, \
             tc.tile_pool(name="ed2ps", bufs=2, space="PSUM") as ed2psp, \
             tc.tile_pool(name="agg2ps", bufs=6, space="PSUM") as agg2psp, \
             tc.tile_pool(name="o2", bufs=2) as o2p:
            for g, (gm0, gm1) in enumerate(groupsD):
                nt = ntileD[g]
                ko = koff[g]
                t0 = int(tbase[g])
                sw0, sw1 = segW_g0[g], segW_g0[g + 1]
                sm0, sm1 = segM_g0[g], segM_g0[g + 1]
                g2 = g2p.tile([128, GTD * ROW2], FP16, tag="g2")
                g2v = g2[:].rearrange("p (t c) -> p t c", c=ROW2)
                nc.gpsimd.dma_gather(
                    g2v[:, 0:ko, :], tab_lo,
                    ixlo[:, glo_off[g]:glo_off[g] + ko * 8],
                    ko * 128, ko * 128, ROW2, single_packet=False)
                nc.gpsimd.dma_gather(
                    g2v[:, ko:nt, :], tab_hi,
                    ixhi[:, ghi_off[g]:ghi_off[g] + (nt - ko) * 8],
                    (nt - ko) * 128, (nt - ko) * 128, ROW2,
                    single_packet=False)
                pt2 = pt2p.tile([128, GSW * WIN], FP16, tag="pt2")
                nc.sync.dma_start(pt2[:, 0:(sw1 - sw0) * WIN],
                                  pat2_d.ap()[:, sw0 * WIN:sw1 * WIN])
                ptt2 = ptt2p.tile([128, GSM * 128], FP16, tag="ptt2")
                nc.sync.dma_start(ptt2[:, 0:(sm1 - sm0) * 128],
                                  patT2_d.ap()[:, sm0 * 128:sm1 * 128])
                if g == 0:
                    nc.sync.dma_start(dbg_g2_d.ap(), g2[:, 0:8 * ROW2])
                    nc.sync.dma_start(dbg_d2_d.ap(), d2_all[:])
                # dst dots
                ed2 = ed2psp.tile([128, GTD], FP32, space="PSUM", tag="ed2")
                for i in range(sm0, sm1):
                    t, mac, st_, sp_ = segM[i]
                    j = t - t0
                    nc.tensor.matmul(ed2[:, j:j + 1],
                                     ptt2[:, (i - sm0) * 128:(i - sm0 + 1) * 128],
                                     d2_all[:, mac:mac + 1],
                                     start=st_, stop=sp_)
                z2 = zu2p.tile([128, GTD], FP32, tag="z2")
                nc.vector.tensor_tensor(
                    out=z2[:, 0:nt].unsqueeze(2),
                    in0=g2v[:, 0:nt, 33:34],
                    in1=ed2[:, 0:nt].unsqueeze(2),
                    op=mybir.AluOpType.add)
                zs2 = zu2p.tile([128, GTD], FP32, tag="zs2")
                nc.vector.tensor_scalar_mul(zs2[:, 0:nt], z2[:, 0:nt], 0.2)
                nc.vector.tensor_tensor(out=z2[:, 0:nt], in0=z2[:, 0:nt],
                                        in1=zs2[:, 0:nt],
                                        op=mybir.AluOpType.max)
                if g == 0:
                    ed2dbg = zu2p.tile([128, 48], FP32, tag="ed2dbg")
                    nc.vector.tensor_copy(ed2dbg[:], ed2[:, 0:48])
                    nc.sync.dma_start(dbg_ed2_d.ap(), ed2dbg[:])
                u2 = zu2p.tile([128, GTD], FP16, tag="u2")
                nc.scalar.activation(u2[:, 0:nt], z2[:, 0:nt],
                                     mybir.ActivationFunctionType.Exp,
                                     bias=nsh2[:])
                msg2 = msg2p.tile([128, GTD * F2], FP16, tag="m2")
                nc.vector.tensor_tensor(
                    out=msg2[:, 0:nt * F2]
                        .rearrange("p (t c) -> p t c", c=F2),
                    in0=g2v[:, 0:nt, 0:33],
                    in1=u2[:, 0:nt].unsqueeze(2).broadcast_to([128, nt, 33]),
                    op=mybir.AluOpType.mult)
                # aggregate
                aggm = {}
                for mac in range(gm0, gm1):
                    aggm[mac] = agg2psp.tile([128, F2], FP32, space="PSUM",
                                             tag="ag2", name=f"ag2_{mac}")
                for i in range(sw0, sw1):
                    t, mac, w, st_, sp_ = segW[i]
                    j = t - t0
                    nc.tensor.matmul(
                        aggm[mac][w * WIN:(w + 1) * WIN, :],
                        pt2[:, (i - sw0) * WIN:(i - sw0 + 1) * WIN],
                        msg2[:, j * F2:(j + 1) * F2],
                        start=st_, stop=sp_, tile_position=(0, w * WIN))
                for mac in range(gm0, gm1):
                    r2 = o2p.tile([128, 1], FP32, tag="r2")
                    nc.vector.reciprocal(r2[:], aggm[mac][:, 32:33])
                    o2 = o2p.tile([128, C2], FP32, tag="o2")
                    nc.vector.tensor_tensor(
                        out=o2[:], in0=aggm[mac][:, 0:C2],
                        in1=r2[:].broadcast_to([128, C2]),
                        op=mybir.AluOpType.mult)
                    nc.sync.dma_start(
                        out2_d.ap()[mac * 128:(mac + 1) * 128, :], o2[:])

    nc.compile()
    return nc


_CACHE = {}


def run(inputs, trace=False):
    sched, perB, perD = host_prep(inputs)
    in_maps = make_in_maps(inputs, sched, perB, perD)
    key = (sched["T1"], sched["T2"], sched["NSW"], sched["NSM"])
    if key not in _CACHE:
        _CACHE[key] = build_program(sched)
    nc = _CACHE[key]
    res = run_bass_kernel_spmd(nc, in_maps, core_ids=list(range(NCORE)),
                               trace=trace)
    out = np.zeros((N, C2), np.float32)
    for c in range(NCORE):
        lo = c * CS
        hi = min(lo + CS, N)
        out[lo:hi] = res.results[c]["out2"][: hi - lo]
    return out, res


def kernel(**inputs):
    out, _res = run(inputs)
    return out.astype(np.float32)
